# revision 61
# baseline (speedup 1.0000x reference)
"""Trainium2 Bass kernel for nn_GatedAttn (gated attention with TISA bias).

Takes FULL inputs, returns FULL output. 8 NeuronCores, sharded as
(batch b = core//4) x (query-row slice r = core%4, 512 rows each); each core
runs the whole pipeline for its 512 query rows (K^T/V projections are
recomputed per core -- an AllGather variant that shares them across the
batch's 4 cores was measured SLOWER: the DRAM-DRAM collective exposes
~130us of latency that the saved PE time cannot cover).

Queries are processed in REVERSED order (host feeds xq columns backwards and
un-reverses output rows) so the per-k-block TISA bias factor is an ascending
contiguous slice of the eu table -- a descending slice would cost one 2-byte
DMA descriptor per element (5M packets = 5.6 ms, the original bottleneck).

Per-core pipeline (all projection/attention matmuls in fp16 operands with
fp32 PSUM accumulation; rel err ~1.4e-3 vs the 2e-2 gate):
  u-tables:  u[h,y] = sum_k amp*exp(-sh*(y-(511+512r+off))^2) via ACT
             Square/Exp + an amplitude-selector matmul; eu = exp(u) (fp16) to
             DRAM; per head-pair load EB[p,x'] = eu[h, p+x'] (all strides +1).
  proj:      Q^T upfront from xq (the core's 512 columns of x^T); K^T and
             V computed just-in-time per head pair / 4-head group, with the
             proj matmuls software-pipelined into the previous pair's
             attention loop. V gets a ones column per head (V_aug, 65-wide)
             so the AV matmul also produces softmax row sums for free.
  attention: scores^T tiles (k_pos x q) via QK matmuls (contraction hd=64,
             head pairs at base partitions 0/64). Softmax without
             max-subtraction (|score| <= ~8.1): ACT exp (PSUM f32 -> SBUF
             fp16), DVE 2x-mode fp16 multiply with the EB table, fp16 AV
             matmuls; attn^T accumulates over 16 k-blocks in PSUM, row 64 =
             denominators. Denominator rows are staged to a [16,512] tile;
             ONE batched DVE reciprocal (a [1,512] reciprocal is 4.3us of
             single-lane 8-cycle divide), then per pair one selector matmul
             broadcasts both heads' reciprocals to 128 rows + one DVE mult.
  gate:      (512 q x 2048) = out^T @ w_gate + b_gate (K=1 ones matmul),
             a * sigmoid(g) -> (512, 1024) output slice.

fp32r/fp16 PSUM-accumulation hazard: accumulating matmuls into a bank need
>=3 intervening matmuls -> all accumulation loops rotate >=4 bank targets.
"""

import sys
import os

for _p in ("/opt/trn_rl_repo", "/opt/pypackages"):
    if os.path.isdir(_p) and _p not in sys.path:
        sys.path.append(_p)

import numpy as np

import concourse.bass as bass
from concourse import bacc
import concourse.mybir as mybir
from concourse.tile import TileContext
from concourse.bass_utils import run_bass_kernel_spmd

F32 = mybir.dt.float32
F16 = mybir.dt.float16
F32R = mybir.dt.float32r
I32 = mybir.dt.int32
AF = mybir.ActivationFunctionType
MULT = mybir.AluOpType.mult
ADD = mybir.AluOpType.add

B, S, D = 2, 2048, 1024
H, NK, HD = 16, 21, 64
QS = 512
NCORES = 8
NPAIR = H // 2
NKB = S // 128
EBW = 2432
EUW = 2560
GROUPS = ((0, 112), (112, 112), (224, 112))


def build(debug=False):
    nc = bacc.Bacc("TRN2", target_bir_lowering=False, debug=False)

    xT_d = nc.dram_tensor("xT", [D, S], F16, kind="ExternalInput")
    xq_d = nc.dram_tensor("xq", [D, QS], F16, kind="ExternalInput")
    w_in_d = nc.dram_tensor("w_in", [D, 3 * D], F16, kind="ExternalInput")
    w_gate_d = nc.dram_tensor("w_gate", [D, 2 * D], F16, kind="ExternalInput")
    b_gate_d = nc.dram_tensor("b_gate", [1, 2 * D], F32R, kind="ExternalInput")
    eamp_d = nc.dram_tensor("eamp", [336, 16], F32R, kind="ExternalInput")
    offp_d = nc.dram_tensor("offp", [336, 1], F32, kind="ExternalInput")
    shp_d = nc.dram_tensor("shp", [336, 1], F32, kind="ExternalInput")
    rsh_d = nc.dram_tensor("rsh", [1, 1], F32, kind="ExternalInput")
    ones_d = nc.dram_tensor("ones", [1, 128], F32R, kind="ExternalInput")
    sel_d = nc.dram_tensor("sel", [16, 1024], F32R, kind="ExternalInput")

    out_d = nc.dram_tensor("out", [QS, D], F32, kind="ExternalOutput")
    eu_dram = nc.dram_tensor("eu_scratch", [H, EUW], F16)
    if debug:
        eu_dbg = nc.dram_tensor("eu_dbg", [H, EUW], F16, kind="ExternalOutput")
        qt_dbg = nc.dram_tensor("qt_dbg", [128, QS], F32R, kind="ExternalOutput")
        kp_dbg = nc.dram_tensor("kp_dbg", [128, S], F32R, kind="ExternalOutput")
        wt_dbg = nc.dram_tensor("wt_dbg", [128, 1024], F32R, kind="ExternalOutput")
        po_dbg = nc.dram_tensor("po_dbg", [65, 1024], F32, kind="ExternalOutput")
        v_dbg = nc.dram_tensor("v_dbg", [128, 260], F32R, kind="ExternalOutput")

    with TileContext(nc) as tc:
        with tc.tile_pool(name="gpool", bufs=1) as gpool:
          with tc.tile_pool(name="psum_m", bufs=3, space="PSUM") as psm, \
               tc.tile_pool(name="psum_o", bufs=2, space="PSUM") as pso:
            ones_sb = gpool.tile([1, 128], F32R, name="ones_sb")
            nc.sync.dma_start(ones_sb[:, :], ones_d[:, :])
            sel_sb = gpool.tile([16, 1024], F32R, name="sel_sb")
            nc.sync.dma_start(sel_sb[:, :], sel_d[:, :])
            # xq (dies after Q proj) shares tags with outT (written later)
            xq_sb = [gpool.tile([128, QS], F16, name=f"xq{d}", tag=f"xo{d}")
                     for d in range(8)]
            for d in range(8):
                nc.sync.dma_start(xq_sb[d][:, :], xq_d[128 * d:128 * d + 128, :])

            # ============ TISA tables ============
            with tc.tile_pool(name="upool", bufs=2) as upool:
                eu_ps_t = [psm.tile([128, 1024], F32, name=f"eups{i}", tag="pm")
                           for i in range(3)]
                eu_ps = [eu_ps_t[c // 2][0:16, 512 * (c % 2):512 * (c % 2) + 512]
                         for c in range(5)]
                # iota strips are identical for every amplitude group: hoist
                # them (15 -> 5 gpsimd ops; each costs ~1.2us serial)
                iofs = []
                for c in range(5):
                    t = upool.tile([112, 512], F32, name=f"iof{c}",
                                   tag=f"u8{c}", bufs=1)
                    nc.gpsimd.iota(t[:, :], pattern=[[1, 512]], base=512 * c,
                                   channel_multiplier=0,
                                   allow_small_or_imprecise_dtypes=True)
                    iofs.append(t)
                for gi, (g0, grows) in enumerate(GROUPS):
                    offg = upool.tile([112, 1], F32, name="offg", tag="u1")
                    nc.sync.dma_start(offg[:, :], offp_d[g0:g0 + 112, :])
                    rshg = upool.tile([112, 1], F32, name="rshg", tag="u2")
                    nc.sync.dma_start(rshg[:, :],
                                      bass.AP(rsh_d, 0, [[0, 112], [1, 1]]))
                    shg = upool.tile([112, 1], F32, name="shg", tag="u3")
                    nc.sync.dma_start(shg[:, :], shp_d[g0:g0 + 112, :])
                    negP = upool.tile([112, 1], F32, name="negP", tag="u4")
                    nc.vector.tensor_tensor(negP[:, :], offg[:, :], rshg[:, :], ADD)
                    nc.vector.tensor_scalar_mul(negP[:, :], negP[:, :], -1.0)
                    negsh = upool.tile([112, 1], F32, name="negsh", tag="u5")
                    nc.vector.tensor_scalar_mul(negsh[:, :], shg[:, :], -1.0)
                    Eg = upool.tile([112, 16], F32R, name="Eg", tag="u6")
                    nc.sync.dma_start(Eg[:, :], eamp_d[g0:g0 + 112, :])
                    for c in range(5):  # 512-wide strips of y
                        sqg = upool.tile([112, 512], F32, name="sqg", tag="u9")
                        nc.scalar.activation(sqg[:, :], iofs[c][:, :], AF.Square,
                                             bias=negP[:, 0:1])
                        Gg = upool.tile([112, 512], F32R, name="Gg", tag="u10")
                        nc.scalar.activation(Gg[:, :], sqg[:, :], AF.Exp,
                                             scale=negsh[:, 0:1])
                        nc.tensor.matmul(eu_ps[c], Eg[:, :], Gg[:, :],
                                         start=(gi == 0), stop=(gi == 2),
                                         skip_group_check=True)
                for c in range(5):
                    eu_sb = upool.tile([16, 512], F16, name="eu_sb", tag="u11")
                    nc.scalar.activation(eu_sb[:, :], eu_ps[c], AF.Exp)
                    nc.sync.dma_start(eu_dram[:, 512 * c:512 * c + 512],
                                      eu_sb[:, :])
                    if debug:
                        nc.sync.dma_start(eu_dbg[:, 512 * c:512 * c + 512],
                                          eu_sb[:, :])

            # ============ main span ============
            with tc.tile_pool(name="span", bufs=1) as span, \
                 tc.tile_pool(name="strm", bufs=2) as strm:

                xT_sb = [span.tile([128, S], F16, name=f"xT{d}")
                         for d in range(8)]
                for d in range(8):
                    nc.sync.dma_start(xT_sb[d][:, :],
                                      xT_d[128 * d:128 * d + 128, :])

                # ---- Q^T proj (upfront) ----
                qT_sb = [span.tile([128, QS], F16, name=f"qT{cb}")
                         for cb in range(8)]
                for quad in range(2):
                    pq = [psm.tile([128, 1024], F32, name=f"pq{quad}{t}",
                                   tag="pm") for t in range(2)]
                    tgt = [pq[0][:, 0:512], pq[0][:, 512:1024],
                           pq[1][:, 0:512], pq[1][:, 512:1024]]
                    for d in range(8):
                        wq = strm.tile([128, 512], F16, name="wq", tag="w",
                                       bufs=8)
                        nc.sync.dma_start(
                            wq[:, :],
                            w_in_d[128 * d:128 * d + 128,
                                   2 * D + 512 * quad:2 * D + 512 * quad + 512])
                        for t in range(4):
                            nc.tensor.matmul(
                                tgt[t], wq[:, 128 * t:128 * t + 128],
                                xq_sb[d][:, :], start=(d == 0), stop=(d == 7),
                                skip_group_check=True)
                    for t in range(4):
                        nc.vector.tensor_copy(qT_sb[4 * quad + t][:, :], tgt[t])

                # ---- attention with JIT K/V proj ----
                # one slot per group (not per group-parity): V proj runs at
                # supergroup granularity (2 groups per 512-wide matmul), so
                # two groups are written while up to two others are still live
                v_sb = [[span.tile([128, 260], F16, name=f"v{gg}_{kb}",
                                   tag=f"v{gg}_{kb}")
                         for kb in range(NKB)] for gg in range(4)]
                kp_pool = [span.tile([128, S], F16, name=f"kp{i}")
                           for i in range(2)]
                outT_sb = [gpool.tile([128, QS], F16, name=f"outT{p}",
                                      tag=f"xo{p}") for p in range(NPAIR)]
                den_sb = span.tile([16, QS], F32, name="den_sb")

                def vproj_ops(sg):
                    # supergroup sg: groups 2sg, 2sg+1 (8 heads) in 512-wide
                    # matmuls -- half the matmul/LDWEIGHTS count of per-group
                    # 256-wide proj, so the weight loads hide under streaming
                    ops = []
                    wv = []

                    def load_wv():
                        for d in range(8):
                            t = strm.tile([128, 512], F16, name="wv", tag="w",
                                          bufs=8)
                            nc.sync.dma_start(
                                t[:, :],
                                w_in_d[128 * d:128 * d + 128,
                                       D + 512 * sg:D + 512 * sg + 512])
                            wv.append(t)
                    ops.append(load_wv)
                    for kq in range(4):
                        def mkv(kq):
                            def f():
                                # each 512-wide target bank-aligned: start=True
                                # clears the whole bank, so no two kb tiles may
                                # share a bank.
                                psv = [psm.tile([128, 1024], F32, name="psv",
                                                tag="pm") for _ in range(2)]
                                tg = [psv[t // 2][:, 512 * (t % 2):
                                                  512 * (t % 2) + 512]
                                      for t in range(4)]
                                for d in range(8):
                                    for t in range(4):
                                        kb = 4 * kq + t
                                        nc.tensor.matmul(
                                            tg[t],
                                            xT_sb[d][:, 128 * kb:128 * kb + 128],
                                            wv[d][:, :], start=(d == 0),
                                            stop=(d == 7), skip_group_check=True)
                                for t in range(4):
                                    kb = 4 * kq + t
                                    for gh in range(2):
                                        g = 2 * sg + gh
                                        dst = v_sb[g][kb][:, :].rearrange(
                                            "p (j w) -> p j w", w=65)[:, :, 0:64]
                                        nc.scalar.activation(
                                            dst,
                                            tg[t][:, 256 * gh:256 * gh + 256]
                                            .rearrange("p (j w) -> p j w", w=64),
                                            AF.Identity)
                                        oc = v_sb[g][kb][:, :].rearrange(
                                            "p (j w) -> p j w", w=65)[:, :, 64:65]
                                        nc.gpsimd.memset(oc, 1.0)
                            return f
                        ops.append(mkv(kq))
                    return ops

                def kproj_ops(p):
                    ops = []
                    kp = kp_pool[p % 2]
                    wk = []

                    def load_wk():
                        for d in range(8):
                            t = strm.tile([128, 128], F16, name="wk", tag="w",
                                          bufs=8)
                            nc.sync.dma_start(
                                t[:, :],
                                w_in_d[128 * d:128 * d + 128,
                                       128 * p:128 * p + 128])
                            wk.append(t)
                    ops.append(load_wk)
                    for half in range(2):
                        def mkk(half):
                            def f():
                                psk = psm.tile([128, 1024], F32, name="psk",
                                               tag="pm")
                                for d in range(8):
                                    for t in range(2):
                                        nc.tensor.matmul(
                                            psk[:, 512 * t:512 * t + 512],
                                            wk[d][:, :],
                                            xT_sb[d][:, 1024 * half + 512 * t:
                                                     1024 * half + 512 * t + 512],
                                            start=(d == 0), stop=(d == 7),
                                            skip_group_check=True)
                                nc.vector.tensor_copy(
                                    kp[:, 1024 * half:1024 * half + 1024],
                                    psk[:, :])
                            return f
                        ops.append(mkk(half))
                    return ops

                # prologue: V supergroup 0 (groups 0+1), K pair 0
                for op in vproj_ops(0):
                    op()
                for op in kproj_ops(0):
                    op()

                # prefetch gate weights during attention: loading them when
                # the gate pool opens (after the span pools close) exposes
                # ~4MB of DMA latency as a tensor-idle dip before the gate.
                wg_sb = [[gpool.tile([128, 1024], F16, name=f"wg{ph}{d}")
                          for d in range(8)] for ph in range(2)]
                for ph in range(2):
                    for d in range(8):
                        nc.sync.dma_start(
                            wg_sb[ph][d][:, :],
                            bass.AP(w_gate_d, 128 * d * 2 * D + 512 * ph,
                                    [[2 * D, 128], [1024, 2], [1, 512]]))

                if debug:
                    nc.sync.dma_start(qt_dbg[:, :], qT_sb[0][:, :])
                    nc.sync.dma_start(kp_dbg[:, :], kp_pool[0][:, :])
                    nc.sync.dma_start(v_dbg[:, :], v_sb[0][0][:, :])
                for p in range(NPAIR):
                    hA = 2 * p
                    g, j0 = p // 2, 2 * (p % 2)
                    kp = kp_pool[p % 2]
                    eb = strm.tile([128, 2 * EBW], F16, name="eb", tag="eb")
                    for hi in range(2):
                        # Queries run REVERSED (host feeds xq columns
                        # backwards), so the bias factor for score tile kb at
                        # [p, j] is eu[h, 128*kb + p + j]: load the diagonal
                        # table as eb[p, x'] = eu[h, p + x'] -- all strides +1
                        # and contiguous (a -1 stride here costs one 2-byte
                        # DMA descriptor per element: 5M packets, 5.6 ms).
                        nc.sync.dma_start(
                            eb[:, EBW * hi:EBW * hi + EBW],
                            bass.AP(eu_dram, (hA + hi) * EUW,
                                    [[1, 128], [1, EBW]]))
                    pend = []
                    if p + 1 < NPAIR:
                        if (p + 1) % 4 == 0:
                            pend += vproj_ops((p + 1) // 4)
                        pend += kproj_ops(p + 1)
                    slots = [[] for _ in range(NKB)]
                    for i, op in enumerate(pend):
                        slots[min(2 + i * 2, NKB - 1)].append(op)

                    po = pso.tile([65, 512], F32, name="po", tag="po")
                    po2 = pso.tile([65, 512], F32, name="po2", tag="po")
                    for kb in range(NKB):
                        psc = psm.tile([128, 1024], F32, name="psc", tag="pm")
                        nc.tensor.matmul(psc[:, 0:512],
                                         kp[0:64, 128 * kb:128 * kb + 128],
                                         qT_sb[p][0:64, :], start=True,
                                         stop=True)
                        nc.tensor.matmul(psc[:, 512:1024],
                                         kp[64:128, 128 * kb:128 * kb + 128],
                                         qT_sb[p][64:128, :], start=True,
                                         stop=True)
                        et = strm.tile([128, 1024], F16, name="et", tag="et",
                                       bufs=3)
                        nc.scalar.activation(et[:, :], psc[:, :], AF.Exp,
                                             scale=0.125)
                        wt = strm.tile([128, 1024], F16, name="wt", tag="wt",
                                       bufs=3)
                        delta = 128 * kb
                        ebv = eb[:, :].rearrange("p (i x) -> p i x", i=2)[
                            :, :, delta:delta + 512]
                        nc.vector.tensor_tensor(
                            wt[:, :].rearrange("p (i q) -> p i q", i=2),
                            et[:, :].rearrange("p (i q) -> p i q", i=2),
                            ebv, MULT)
                        if debug and p == 0 and kb == 5:
                            nc.sync.dma_start(wt_dbg[:, :], wt[:, :])
                        nc.tensor.matmul(
                            po[:, :], v_sb[g][kb][:, 65 * j0:65 * j0 + 65],
                            wt[:, 0:512], start=(kb == 0),
                            stop=(kb == NKB - 1), skip_group_check=True)
                        nc.tensor.matmul(
                            po2[:, :],
                            v_sb[g][kb][:, 65 * (j0 + 1):65 * (j0 + 1) + 65],
                            wt[:, 512:1024], start=(kb == 0),
                            stop=(kb == NKB - 1), skip_group_check=True)
                        for op in slots[kb]:
                            op()
                    for hi, pot in enumerate((po, po2)):
                        if debug and p == 0:
                            pod = strm.tile([65, 512], F32, name="pod", tag="pod")
                            nc.scalar.activation(pod[:, :], pot[:, :], AF.Identity)
                            nc.sync.dma_start(
                                po_dbg[:, 512 * hi:512 * hi + 512], pod[:, :])
                        # stash unnormalized attn + its denominator row; the
                        # reciprocal runs ONCE batched over [16, 512] after the
                        # pair loop (16 single-partition reciprocals = 64us of
                        # 8-cycle/elem DVE divide on one lane).
                        # engines need 32-aligned base partitions, so stage the
                        # denominator row at partition 0 and DMA it into its
                        # den_sb partition (DMA places partitions freely).
                        dstg = strm.tile([1, 512], F32, name="dstg", tag="dstg")
                        nc.scalar.activation(dstg[:, :], pot[64:65, :],
                                             AF.Identity)
                        nc.sync.dma_start(
                            den_sb[2 * p + hi:2 * p + hi + 1, :], dstg[:, :])
                        nc.vector.tensor_copy(
                            outT_sb[p][64 * hi:64 * hi + 64, :], pot[0:64, :])

                # ---- batched softmax normalization ----
                rden = span.tile([16, QS], F32R, name="rden")
                with nc.allow_low_precision(
                        reason="f32r reciprocal: 1.2e-4 rel is fine"):
                    nc.vector.reciprocal(rden[:, :], den_sb[:, :])
                for p in range(NPAIR):
                    pbt = psm.tile([128, 1024], F32, name="pbt", tag="pm")
                    pb = pbt[:, 0:512]
                    # sel block p: pb[c, q] = rden[2p + c//64, q]
                    nc.tensor.matmul(pb, sel_sb[:, 128 * p:128 * p + 128],
                                     rden[:, :], start=True, stop=True)
                    nc.vector.tensor_tensor(
                        outT_sb[p][:, :], outT_sb[p][:, :], pb, MULT)

          # ============ gate + GLU ============
          with tc.tile_pool(name="gate", bufs=2) as gp, \
               tc.tile_pool(name="psum_g", bufs=1, space="PSUM") as psg:
              bg_sb = gp.tile([1, 2 * D], F32R, name="bg_sb", bufs=1)
              nc.sync.dma_start(bg_sb[:, :], b_gate_d[:, :])
              for ph in range(2):
                  pgt = [psg.tile([128, 512], F32, name=f"pg{ph}{i}",
                                  tag=f"pg{i}") for i in range(8)]
                  for d in range(8):
                      wg = wg_sb[ph][d]
                      for qb in range(4):
                          for ci in range(2):
                              nc.tensor.matmul(
                                  pgt[2 * qb + ci],
                                  outT_sb[d][:, 128 * qb:128 * qb + 128],
                                  wg[:, 512 * ci:512 * ci + 512],
                                  start=(d == 0), stop=False,
                                  skip_group_check=True)
                  for qb in range(4):
                      for ci in range(2):
                          nc.tensor.matmul(
                              pgt[2 * qb + ci], ones_sb[:, :],
                              bg_sb[:, 1024 * ci + 512 * ph:
                                    1024 * ci + 512 * ph + 512],
                              start=False, stop=True, skip_group_check=True)
                  for qb in range(4):
                      sg = gp.tile([128, 512], F32, name="sg", tag="sg")
                      nc.scalar.activation(sg[:, :], pgt[2 * qb + 1], AF.Sigmoid)
                      res = gp.tile([128, 512], F32, name="res", tag="res")
                      nc.vector.tensor_tensor(res[:, :], pgt[2 * qb], sg[:, :],
                                              MULT)
                      nc.sync.dma_start(
                          out_d[128 * qb:128 * qb + 128,
                                512 * ph:512 * ph + 512],
                          res[:, :])

    nc.finalize()
    return nc


_NC_CACHE = None
_LAST_IN_MAPS = None


def _get_nc():
    global _NC_CACHE
    if _NC_CACHE is None:
        _NC_CACHE = build()
    return _NC_CACHE


def kernel(x, w_in, w_gate, b_gate, amplitudes, sharpness, offsets):
    x = np.ascontiguousarray(x, dtype=np.float32)
    w_in16 = np.ascontiguousarray(w_in, dtype=np.float16)
    w_gate16 = np.ascontiguousarray(w_gate, dtype=np.float16)
    b_gate = np.ascontiguousarray(b_gate, dtype=np.float32).reshape(1, 2 * D)
    amplitudes = np.asarray(amplitudes, dtype=np.float32)
    sharpness = np.asarray(sharpness, dtype=np.float32)
    offsets = np.asarray(offsets, dtype=np.float32)

    eamp = np.zeros((H * NK, 16), np.float32)
    eamp[np.arange(H * NK), np.arange(H * NK) // NK] = amplitudes.reshape(-1)
    offp = offsets.reshape(H * NK, 1)
    shp = sharpness.reshape(H * NK, 1)
    ones = np.ones((1, 128), np.float32)
    # sel[r, 128p + c] = 1 iff r == 2p + c//64 (head selector used to
    # broadcast the batched softmax reciprocals to 128 output rows per pair)
    sel = np.zeros((16, 1024), np.float32)
    for p_ in range(8):
        sel[2 * p_, 128 * p_:128 * p_ + 64] = 1.0
        sel[2 * p_ + 1, 128 * p_ + 64:128 * p_ + 128] = 1.0

    in_maps = []
    for c in range(NCORES):
        b, r = c // 4, c % 4
        xT = np.ascontiguousarray(x[b].T, dtype=np.float16)
        # query columns fed in REVERSED order so the TISA bias slice per
        # k-block is an ascending (contiguous-DMA) slice of the eu table;
        # the output rows are un-reversed after the run.
        xq = np.ascontiguousarray(x[b, QS * r:QS * r + QS, :].T[:, ::-1],
                                  dtype=np.float16)
        rsh = np.array([[511.0 + 512.0 * r]], np.float32)
        in_maps.append({
            "xT": xT, "xq": xq, "w_in": w_in16, "w_gate": w_gate16,
            "b_gate": b_gate, "eamp": eamp, "offp": offp, "shp": shp,
            "rsh": rsh, "ones": ones, "sel": sel,
        })

    global _LAST_IN_MAPS
    _LAST_IN_MAPS = in_maps
    nc = _get_nc()
    r_ = run_bass_kernel_spmd(nc, in_maps, core_ids=list(range(NCORES)))
    out = np.empty((B, S, D), np.float32)
    for c in range(NCORES):
        b, r = c // 4, c % 4
        out[b, QS * r:QS * r + QS, :] = r_.results[c]["out"][::-1, :]
    return out



# revision 62
# speedup vs baseline: 1.0339x; 1.0339x over previous
"""Trainium2 Bass kernel for nn_GatedAttn (gated attention with TISA bias).

Takes FULL inputs, returns FULL output. 8 NeuronCores, sharded as
(batch b = core//4) x (query-row slice r = core%4, 512 rows each); each core
runs the whole pipeline for its 512 query rows (K^T/V projections are
recomputed per core -- an AllGather variant that shares them across the
batch's 4 cores was measured SLOWER: the DRAM-DRAM collective exposes
~130us of latency that the saved PE time cannot cover).

Queries are processed in REVERSED order (host feeds xq columns backwards and
un-reverses output rows) so the per-k-block TISA bias factor is an ascending
contiguous slice of the eu table -- a descending slice would cost one 2-byte
DMA descriptor per element (5M packets = 5.6 ms, the original bottleneck).

Per-core pipeline (all projection/attention matmuls in fp16 operands with
fp32 PSUM accumulation; rel err ~1.4e-3 vs the 2e-2 gate):
  u-tables:  u[h,y] = sum_k amp*exp(-sh*(y-(511+512r+off))^2) via ACT
             Square/Exp + an amplitude-selector matmul; eu = exp(u) (fp16) to
             DRAM; per head-pair load EB[p,x'] = eu[h, p+x'] (all strides +1).
  proj:      Q^T upfront from xq (the core's 512 columns of x^T); K^T and
             V computed just-in-time per head pair / 4-head group, with the
             proj matmuls software-pipelined into the previous pair's
             attention loop. V gets a ones column per head (V_aug, 65-wide)
             so the AV matmul also produces softmax row sums for free.
  attention: scores^T tiles (k_pos x q) via QK matmuls (contraction hd=64,
             head pairs at base partitions 0/64). Softmax without
             max-subtraction (|score| <= ~8.1): ACT exp (PSUM f32 -> SBUF
             fp16), DVE 2x-mode fp16 multiply with the EB table, fp16 AV
             matmuls; attn^T accumulates over 16 k-blocks in PSUM, row 64 =
             denominators. Denominator rows are staged to a [16,512] tile;
             ONE batched DVE reciprocal (a [1,512] reciprocal is 4.3us of
             single-lane 8-cycle divide), then per pair one selector matmul
             broadcasts both heads' reciprocals to 128 rows + one DVE mult.
  gate:      (512 q x 2048) = out^T @ w_gate + b_gate (K=1 ones matmul),
             a * sigmoid(g) -> (512, 1024) output slice.

fp32r/fp16 PSUM-accumulation hazard: accumulating matmuls into a bank need
>=3 intervening matmuls -> all accumulation loops rotate >=4 bank targets.
"""

import sys
import os

for _p in ("/opt/trn_rl_repo", "/opt/pypackages"):
    if os.path.isdir(_p) and _p not in sys.path:
        sys.path.append(_p)

import numpy as np

import concourse.bass as bass
from concourse import bacc
import concourse.mybir as mybir
from concourse.tile import TileContext
from concourse.bass_utils import run_bass_kernel_spmd

F32 = mybir.dt.float32
F16 = mybir.dt.float16
F32R = mybir.dt.float32r
I32 = mybir.dt.int32
AF = mybir.ActivationFunctionType
MULT = mybir.AluOpType.mult
ADD = mybir.AluOpType.add

B, S, D = 2, 2048, 1024
H, NK, HD = 16, 21, 64
QS = 512
NCORES = 8
NPAIR = H // 2
NKB = S // 128
EBW = 2432
EUW = 2560
GROUPS = ((0, 112), (112, 112), (224, 112))


def build(debug=False):
    nc = bacc.Bacc("TRN2", target_bir_lowering=False, debug=False)

    xT_d = nc.dram_tensor("xT", [D, S], F16, kind="ExternalInput")
    xq_d = nc.dram_tensor("xq", [D, QS], F16, kind="ExternalInput")
    w_in_d = nc.dram_tensor("w_in", [D, 3 * D], F16, kind="ExternalInput")
    w_gate_d = nc.dram_tensor("w_gate", [D, 2 * D], F16, kind="ExternalInput")
    b_gate_d = nc.dram_tensor("b_gate", [1, 2 * D], F32R, kind="ExternalInput")
    eamp_d = nc.dram_tensor("eamp", [336, 16], F32R, kind="ExternalInput")
    offp_d = nc.dram_tensor("offp", [336, 1], F32, kind="ExternalInput")
    shp_d = nc.dram_tensor("shp", [336, 1], F32, kind="ExternalInput")
    rsh_d = nc.dram_tensor("rsh", [1, 1], F32, kind="ExternalInput")
    ones_d = nc.dram_tensor("ones", [1, 128], F32R, kind="ExternalInput")
    sel_d = nc.dram_tensor("sel", [16, 1024], F32R, kind="ExternalInput")

    out_d = nc.dram_tensor("out", [QS, D], F32, kind="ExternalOutput")
    eu_dram = nc.dram_tensor("eu_scratch", [H, EUW], F16)
    if debug:
        eu_dbg = nc.dram_tensor("eu_dbg", [H, EUW], F16, kind="ExternalOutput")
        qt_dbg = nc.dram_tensor("qt_dbg", [128, QS], F32R, kind="ExternalOutput")
        kp_dbg = nc.dram_tensor("kp_dbg", [128, S], F32R, kind="ExternalOutput")
        wt_dbg = nc.dram_tensor("wt_dbg", [128, 1024], F32R, kind="ExternalOutput")
        po_dbg = nc.dram_tensor("po_dbg", [65, 1024], F32, kind="ExternalOutput")
        v_dbg = nc.dram_tensor("v_dbg", [128, 260], F32R, kind="ExternalOutput")

    with TileContext(nc) as tc:
        with tc.tile_pool(name="gpool", bufs=1) as gpool:
          with tc.tile_pool(name="psum_m", bufs=3, space="PSUM") as psm, \
               tc.tile_pool(name="psum_o", bufs=2, space="PSUM") as pso:
            ones_sb = gpool.tile([1, 128], F32R, name="ones_sb")
            nc.sync.dma_start(ones_sb[:, :], ones_d[:, :])
            sel_sb = gpool.tile([16, 1024], F32R, name="sel_sb")
            nc.sync.dma_start(sel_sb[:, :], sel_d[:, :])
            # xq (dies after Q proj) shares tags with outT (written later)
            xq_sb = [gpool.tile([128, QS], F16, name=f"xq{d}", tag=f"xo{d}")
                     for d in range(8)]
            for d in range(8):
                nc.sync.dma_start(xq_sb[d][:, :], xq_d[128 * d:128 * d + 128, :])

            # ============ TISA tables ============
            with tc.tile_pool(name="upool", bufs=2) as upool:
                eu_ps_t = [psm.tile([128, 1024], F32, name=f"eups{i}", tag="pm")
                           for i in range(3)]
                eu_ps = [eu_ps_t[c // 2][0:16, 512 * (c % 2):512 * (c % 2) + 512]
                         for c in range(5)]
                # iota strips are identical for every amplitude group: hoist
                # them (15 -> 5 gpsimd ops; each costs ~1.2us serial)
                iofs = []
                for c in range(5):
                    t = upool.tile([112, 512], F32, name=f"iof{c}",
                                   tag=f"u8{c}", bufs=1)
                    nc.gpsimd.iota(t[:, :], pattern=[[1, 512]], base=512 * c,
                                   channel_multiplier=0,
                                   allow_small_or_imprecise_dtypes=True)
                    iofs.append(t)
                for gi, (g0, grows) in enumerate(GROUPS):
                    offg = upool.tile([112, 1], F32, name="offg", tag="u1")
                    nc.sync.dma_start(offg[:, :], offp_d[g0:g0 + 112, :])
                    rshg = upool.tile([112, 1], F32, name="rshg", tag="u2")
                    nc.sync.dma_start(rshg[:, :],
                                      bass.AP(rsh_d, 0, [[0, 112], [1, 1]]))
                    shg = upool.tile([112, 1], F32, name="shg", tag="u3")
                    nc.sync.dma_start(shg[:, :], shp_d[g0:g0 + 112, :])
                    negP = upool.tile([112, 1], F32, name="negP", tag="u4")
                    nc.vector.tensor_tensor(negP[:, :], offg[:, :], rshg[:, :], ADD)
                    nc.vector.tensor_scalar_mul(negP[:, :], negP[:, :], -1.0)
                    negsh = upool.tile([112, 1], F32, name="negsh", tag="u5")
                    nc.vector.tensor_scalar_mul(negsh[:, :], shg[:, :], -1.0)
                    Eg = upool.tile([112, 16], F32R, name="Eg", tag="u6")
                    nc.sync.dma_start(Eg[:, :], eamp_d[g0:g0 + 112, :])
                    for c in range(5):  # 512-wide strips of y
                        sqg = upool.tile([112, 512], F32, name="sqg", tag="u9")
                        nc.scalar.activation(sqg[:, :], iofs[c][:, :], AF.Square,
                                             bias=negP[:, 0:1])
                        Gg = upool.tile([112, 512], F32R, name="Gg", tag="u10")
                        nc.scalar.activation(Gg[:, :], sqg[:, :], AF.Exp,
                                             scale=negsh[:, 0:1])
                        nc.tensor.matmul(eu_ps[c], Eg[:, :], Gg[:, :],
                                         start=(gi == 0), stop=(gi == 2),
                                         skip_group_check=True)
                for c in range(5):
                    eu_sb = upool.tile([16, 512], F16, name="eu_sb", tag="u11")
                    nc.scalar.activation(eu_sb[:, :], eu_ps[c], AF.Exp)
                    nc.sync.dma_start(eu_dram[:, 512 * c:512 * c + 512],
                                      eu_sb[:, :])
                    if debug:
                        nc.sync.dma_start(eu_dbg[:, 512 * c:512 * c + 512],
                                          eu_sb[:, :])

            # ============ main span ============
            with tc.tile_pool(name="span", bufs=1) as span, \
                 tc.tile_pool(name="strm", bufs=2) as strm:

                xT_sb = [span.tile([128, S], F16, name=f"xT{d}")
                         for d in range(8)]
                for d in range(8):
                    nc.sync.dma_start(xT_sb[d][:, :],
                                      xT_d[128 * d:128 * d + 128, :])

                # ---- Q^T proj (upfront) ----
                qT_sb = [span.tile([128, QS], F16, name=f"qT{cb}")
                         for cb in range(8)]
                for quad in range(2):
                    pq = [psm.tile([128, 1024], F32, name=f"pq{quad}{t}",
                                   tag="pm") for t in range(2)]
                    tgt = [pq[0][:, 0:512], pq[0][:, 512:1024],
                           pq[1][:, 0:512], pq[1][:, 512:1024]]
                    for d in range(8):
                        wq = strm.tile([128, 512], F16, name="wq", tag="w",
                                       bufs=8)
                        nc.sync.dma_start(
                            wq[:, :],
                            w_in_d[128 * d:128 * d + 128,
                                   2 * D + 512 * quad:2 * D + 512 * quad + 512])
                        for t in range(4):
                            nc.tensor.matmul(
                                tgt[t], wq[:, 128 * t:128 * t + 128],
                                xq_sb[d][:, :], start=(d == 0), stop=(d == 7),
                                skip_group_check=True)
                    for t in range(4):
                        nc.vector.tensor_copy(qT_sb[4 * quad + t][:, :], tgt[t])

                # ---- attention with JIT K/V proj ----
                v_sb = [[span.tile([128, 260], F16, name=f"v{gg}_{kb}",
                                   tag=f"v{gg % 2}_{kb}")
                         for kb in range(NKB)] for gg in range(4)]
                kp_pool = [span.tile([128, S], F16, name=f"kp{i}")
                           for i in range(2)]
                outT_sb = [gpool.tile([128, QS], F16, name=f"outT{p}",
                                      tag=f"xo{p}") for p in range(NPAIR)]
                den_sb = span.tile([16, QS], F32, name="den_sb")

                def vproj_ops(g):
                    ops = []
                    wv = []

                    def load_wv():
                        for d in range(8):
                            t = strm.tile([128, 256], F16, name="wv", tag="w",
                                          bufs=8)
                            nc.sync.dma_start(
                                t[:, :],
                                w_in_d[128 * d:128 * d + 128,
                                       D + 256 * g:D + 256 * g + 256])
                            wv.append(t)
                    ops.append(load_wv)
                    for kq in range(4):
                        def mkv(kq):
                            def f():
                                # each 256-wide target bank-aligned: start=True
                                # clears the whole bank, so no two kb tiles may
                                # share a bank.
                                psv = [psm.tile([128, 1024], F32, name="psv",
                                                tag="pm") for _ in range(2)]
                                tg = [psv[t // 2][:, 512 * (t % 2):
                                                  512 * (t % 2) + 256]
                                      for t in range(4)]
                                for d in range(8):
                                    for t in range(4):
                                        kb = 4 * kq + t
                                        nc.tensor.matmul(
                                            tg[t],
                                            xT_sb[d][:, 128 * kb:128 * kb + 128],
                                            wv[d][:, :], start=(d == 0),
                                            stop=(d == 7), skip_group_check=True)
                                for t in range(4):
                                    kb = 4 * kq + t
                                    dst = v_sb[g][kb][:, :].rearrange(
                                        "p (j w) -> p j w", w=65)[:, :, 0:64]
                                    nc.scalar.activation(
                                        dst,
                                        tg[t].rearrange("p (j w) -> p j w", w=64),
                                        AF.Identity)
                                    oc = v_sb[g][kb][:, :].rearrange(
                                        "p (j w) -> p j w", w=65)[:, :, 64:65]
                                    nc.gpsimd.memset(oc, 1.0)
                            return f
                        ops.append(mkv(kq))
                    return ops

                def kproj_ops(p):
                    ops = []
                    kp = kp_pool[p % 2]
                    wk = []

                    def load_wk():
                        for d in range(8):
                            t = strm.tile([128, 128], F16, name="wk", tag="w",
                                          bufs=8)
                            nc.sync.dma_start(
                                t[:, :],
                                w_in_d[128 * d:128 * d + 128,
                                       128 * p:128 * p + 128])
                            wk.append(t)
                    ops.append(load_wk)
                    for half in range(2):
                        def mkk(half):
                            def f():
                                psk = psm.tile([128, 1024], F32, name="psk",
                                               tag="pm")
                                for d in range(8):
                                    for t in range(2):
                                        nc.tensor.matmul(
                                            psk[:, 512 * t:512 * t + 512],
                                            wk[d][:, :],
                                            xT_sb[d][:, 1024 * half + 512 * t:
                                                     1024 * half + 512 * t + 512],
                                            start=(d == 0), stop=(d == 7),
                                            skip_group_check=True)
                                nc.vector.tensor_copy(
                                    kp[:, 1024 * half:1024 * half + 1024],
                                    psk[:, :])
                            return f
                        ops.append(mkk(half))
                    return ops

                # prologue: V group 0, K pair 0
                for op in vproj_ops(0):
                    op()
                for op in kproj_ops(0):
                    op()

                # prefetch gate weights during attention: loading them when
                # the gate pool opens (after the span pools close) exposes
                # ~4MB of DMA latency as a tensor-idle dip before the gate.
                wg_sb = [[gpool.tile([128, 1024], F16, name=f"wg{ph}{d}")
                          for d in range(8)] for ph in range(2)]
                for ph in range(2):
                    for d in range(8):
                        nc.sync.dma_start(
                            wg_sb[ph][d][:, :],
                            bass.AP(w_gate_d, 128 * d * 2 * D + 512 * ph,
                                    [[2 * D, 128], [1024, 2], [1, 512]]))

                if debug:
                    nc.sync.dma_start(qt_dbg[:, :], qT_sb[0][:, :])
                    nc.sync.dma_start(kp_dbg[:, :], kp_pool[0][:, :])
                    nc.sync.dma_start(v_dbg[:, :], v_sb[0][0][:, :])
                for p in range(NPAIR):
                    hA = 2 * p
                    g, j0 = p // 2, 2 * (p % 2)
                    kp = kp_pool[p % 2]
                    eb = strm.tile([128, 2 * EBW], F16, name="eb", tag="eb")
                    for hi in range(2):
                        # Queries run REVERSED (host feeds xq columns
                        # backwards), so the bias factor for score tile kb at
                        # [p, j] is eu[h, 128*kb + p + j]: load the diagonal
                        # table as eb[p, x'] = eu[h, p + x'] -- all strides +1
                        # and contiguous (a -1 stride here costs one 2-byte
                        # DMA descriptor per element: 5M packets, 5.6 ms).
                        nc.sync.dma_start(
                            eb[:, EBW * hi:EBW * hi + EBW],
                            bass.AP(eu_dram, (hA + hi) * EUW,
                                    [[1, 128], [1, EBW]]))
                    pend = []
                    if p + 1 < NPAIR:
                        if (p + 1) % 2 == 0:
                            pend += vproj_ops((p + 1) // 2)
                        pend += kproj_ops(p + 1)
                    slots = [[] for _ in range(NKB)]
                    for i, op in enumerate(pend):
                        slots[min(2 + i * 2, NKB - 1)].append(op)

                    po = pso.tile([65, 512], F32, name="po", tag="po")
                    po2 = pso.tile([65, 512], F32, name="po2", tag="po")
                    for kb in range(NKB):
                        psc = psm.tile([128, 1024], F32, name="psc", tag="pm")
                        nc.tensor.matmul(psc[:, 0:512],
                                         kp[0:64, 128 * kb:128 * kb + 128],
                                         qT_sb[p][0:64, :], start=True,
                                         stop=True)
                        nc.tensor.matmul(psc[:, 512:1024],
                                         kp[64:128, 128 * kb:128 * kb + 128],
                                         qT_sb[p][64:128, :], start=True,
                                         stop=True)
                        et = strm.tile([128, 1024], F16, name="et", tag="et",
                                       bufs=3)
                        nc.scalar.activation(et[:, :], psc[:, :], AF.Exp,
                                             scale=0.125)
                        wt = strm.tile([128, 1024], F16, name="wt", tag="wt",
                                       bufs=3)
                        delta = 128 * kb
                        ebv = eb[:, :].rearrange("p (i x) -> p i x", i=2)[
                            :, :, delta:delta + 512]
                        nc.vector.tensor_tensor(
                            wt[:, :].rearrange("p (i q) -> p i q", i=2),
                            et[:, :].rearrange("p (i q) -> p i q", i=2),
                            ebv, MULT)
                        if debug and p == 0 and kb == 5:
                            nc.sync.dma_start(wt_dbg[:, :], wt[:, :])
                        nc.tensor.matmul(
                            po[:, :], v_sb[g][kb][:, 65 * j0:65 * j0 + 65],
                            wt[:, 0:512], start=(kb == 0),
                            stop=(kb == NKB - 1), skip_group_check=True)
                        nc.tensor.matmul(
                            po2[:, :],
                            v_sb[g][kb][:, 65 * (j0 + 1):65 * (j0 + 1) + 65],
                            wt[:, 512:1024], start=(kb == 0),
                            stop=(kb == NKB - 1), skip_group_check=True)
                        for op in slots[kb]:
                            op()
                    for hi, pot in enumerate((po, po2)):
                        if debug and p == 0:
                            pod = strm.tile([65, 512], F32, name="pod", tag="pod")
                            nc.scalar.activation(pod[:, :], pot[:, :], AF.Identity)
                            nc.sync.dma_start(
                                po_dbg[:, 512 * hi:512 * hi + 512], pod[:, :])
                        # stash unnormalized attn + its denominator row; the
                        # reciprocal runs ONCE batched over [16, 512] after the
                        # pair loop (16 single-partition reciprocals = 64us of
                        # 8-cycle/elem DVE divide on one lane).
                        # engines need 32-aligned base partitions, so stage the
                        # denominator row at partition 0 and DMA it into its
                        # den_sb partition (DMA places partitions freely).
                        dstg = strm.tile([1, 512], F32, name="dstg", tag="dstg")
                        nc.scalar.activation(dstg[:, :], pot[64:65, :],
                                             AF.Identity)
                        nc.sync.dma_start(
                            den_sb[2 * p + hi:2 * p + hi + 1, :], dstg[:, :])
                        nc.vector.tensor_copy(
                            outT_sb[p][64 * hi:64 * hi + 64, :], pot[0:64, :])

                # ---- batched softmax normalization ----
                rden = span.tile([16, QS], F32R, name="rden")
                with nc.allow_low_precision(
                        reason="f32r reciprocal: 1.2e-4 rel is fine"):
                    nc.vector.reciprocal(rden[:, :], den_sb[:, :])
                for p in range(NPAIR):
                    pbt = psm.tile([128, 1024], F32, name="pbt", tag="pm")
                    pb = pbt[:, 0:512]
                    # sel block p: pb[c, q] = rden[2p + c//64, q]
                    nc.tensor.matmul(pb, sel_sb[:, 128 * p:128 * p + 128],
                                     rden[:, :], start=True, stop=True)
                    nc.vector.tensor_tensor(
                        outT_sb[p][:, :], outT_sb[p][:, :], pb, MULT)

          # ============ gate + GLU ============
          with tc.tile_pool(name="gate", bufs=2) as gp, \
               tc.tile_pool(name="psum_g", bufs=1, space="PSUM") as psg:
              bg_sb = gp.tile([1, 2 * D], F32R, name="bg_sb", bufs=1)
              nc.sync.dma_start(bg_sb[:, :], b_gate_d[:, :])
              for ph in range(2):
                  pgt = [psg.tile([128, 512], F32, name=f"pg{ph}{i}",
                                  tag=f"pg{i}") for i in range(8)]
                  for d in range(8):
                      wg = wg_sb[ph][d]
                      for qb in range(4):
                          for ci in range(2):
                              nc.tensor.matmul(
                                  pgt[2 * qb + ci],
                                  outT_sb[d][:, 128 * qb:128 * qb + 128],
                                  wg[:, 512 * ci:512 * ci + 512],
                                  start=(d == 0), stop=False,
                                  skip_group_check=True)
                  for qb in range(4):
                      for ci in range(2):
                          nc.tensor.matmul(
                              pgt[2 * qb + ci], ones_sb[:, :],
                              bg_sb[:, 1024 * ci + 512 * ph:
                                    1024 * ci + 512 * ph + 512],
                              start=False, stop=True, skip_group_check=True)
                  for qb in range(4):
                      sg = gp.tile([128, 512], F32, name="sg", tag="sg")
                      nc.scalar.activation(sg[:, :], pgt[2 * qb + 1], AF.Sigmoid)
                      res = gp.tile([128, 512], F32, name="res", tag="res")
                      nc.vector.tensor_tensor(res[:, :], pgt[2 * qb], sg[:, :],
                                              MULT)
                      nc.sync.dma_start(
                          out_d[128 * qb:128 * qb + 128,
                                512 * ph:512 * ph + 512],
                          res[:, :])

    nc.finalize()
    return nc


_NC_CACHE = None
_LAST_IN_MAPS = None


def _get_nc():
    global _NC_CACHE
    if _NC_CACHE is None:
        _NC_CACHE = build()
    return _NC_CACHE


def kernel(x, w_in, w_gate, b_gate, amplitudes, sharpness, offsets):
    x = np.ascontiguousarray(x, dtype=np.float32)
    w_in16 = np.ascontiguousarray(w_in, dtype=np.float16)
    w_gate16 = np.ascontiguousarray(w_gate, dtype=np.float16)
    b_gate = np.ascontiguousarray(b_gate, dtype=np.float32).reshape(1, 2 * D)
    amplitudes = np.asarray(amplitudes, dtype=np.float32)
    sharpness = np.asarray(sharpness, dtype=np.float32)
    offsets = np.asarray(offsets, dtype=np.float32)

    eamp = np.zeros((H * NK, 16), np.float32)
    eamp[np.arange(H * NK), np.arange(H * NK) // NK] = amplitudes.reshape(-1)
    offp = offsets.reshape(H * NK, 1)
    shp = sharpness.reshape(H * NK, 1)
    ones = np.ones((1, 128), np.float32)
    # sel[r, 128p + c] = 1 iff r == 2p + c//64 (head selector used to
    # broadcast the batched softmax reciprocals to 128 output rows per pair)
    sel = np.zeros((16, 1024), np.float32)
    for p_ in range(8):
        sel[2 * p_, 128 * p_:128 * p_ + 64] = 1.0
        sel[2 * p_ + 1, 128 * p_ + 64:128 * p_ + 128] = 1.0

    in_maps = []
    for c in range(NCORES):
        b, r = c // 4, c % 4
        xT = np.ascontiguousarray(x[b].T, dtype=np.float16)
        # query columns fed in REVERSED order so the TISA bias slice per
        # k-block is an ascending (contiguous-DMA) slice of the eu table;
        # the output rows are un-reversed after the run.
        xq = np.ascontiguousarray(x[b, QS * r:QS * r + QS, :].T[:, ::-1],
                                  dtype=np.float16)
        rsh = np.array([[511.0 + 512.0 * r]], np.float32)
        in_maps.append({
            "xT": xT, "xq": xq, "w_in": w_in16, "w_gate": w_gate16,
            "b_gate": b_gate, "eamp": eamp, "offp": offp, "shp": shp,
            "rsh": rsh, "ones": ones, "sel": sel,
        })

    global _LAST_IN_MAPS
    _LAST_IN_MAPS = in_maps
    nc = _get_nc()
    r_ = run_bass_kernel_spmd(nc, in_maps, core_ids=list(range(NCORES)))
    out = np.empty((B, S, D), np.float32)
    for c in range(NCORES):
        b, r = c // 4, c % 4
        out[b, QS * r:QS * r + QS, :] = r_.results[c]["out"][::-1, :]
    return out



# revision 63
# speedup vs baseline: 1.0390x; 1.0050x over previous
"""Trainium2 Bass kernel for nn_GatedAttn (gated attention with TISA bias).

Takes FULL inputs, returns FULL output. 8 NeuronCores, sharded as
(batch b = core//4) x (query-row slice r = core%4, 512 rows each); each core
runs the whole pipeline for its 512 query rows (K^T/V projections are
recomputed per core -- an AllGather variant that shares them across the
batch's 4 cores was measured SLOWER: the DRAM-DRAM collective exposes
~130us of latency that the saved PE time cannot cover).

Queries are processed in REVERSED order (host feeds xq columns backwards and
un-reverses output rows) so the per-k-block TISA bias factor is an ascending
contiguous slice of the eu table -- a descending slice would cost one 2-byte
DMA descriptor per element (5M packets = 5.6 ms, the original bottleneck).

Per-core pipeline (all projection/attention matmuls in fp16 operands with
fp32 PSUM accumulation; rel err ~1.4e-3 vs the 2e-2 gate):
  u-tables:  u[h,y] = sum_k amp*exp(-sh*(y-(511+512r+off))^2) via ACT
             Square/Exp + an amplitude-selector matmul; eu = exp(u) (fp16) to
             DRAM; per head-pair load EB[p,x'] = eu[h, p+x'] (all strides +1).
  proj:      Q^T upfront from xq (the core's 512 columns of x^T); K^T and
             V computed just-in-time per head pair / 4-head group, with the
             proj matmuls software-pipelined into the previous pair's
             attention loop. V gets a ones column per head (V_aug, 65-wide)
             so the AV matmul also produces softmax row sums for free.
  attention: scores^T tiles (k_pos x q) via QK matmuls (contraction hd=64,
             head pairs at base partitions 0/64). Softmax without
             max-subtraction (|score| <= ~8.1): ACT exp (PSUM f32 -> SBUF
             fp16), DVE 2x-mode fp16 multiply with the EB table, fp16 AV
             matmuls; attn^T accumulates over 16 k-blocks in PSUM, row 64 =
             denominators. Denominator rows are staged to a [16,512] tile;
             ONE batched DVE reciprocal (a [1,512] reciprocal is 4.3us of
             single-lane 8-cycle divide), then per pair one selector matmul
             broadcasts both heads' reciprocals to 128 rows + one DVE mult.
  gate:      (512 q x 2048) = out^T @ w_gate + b_gate (K=1 ones matmul),
             a * sigmoid(g) -> (512, 1024) output slice.

fp32r/fp16 PSUM-accumulation hazard: accumulating matmuls into a bank need
>=3 intervening matmuls -> all accumulation loops rotate >=4 bank targets.
"""

import sys
import os

for _p in ("/opt/trn_rl_repo", "/opt/pypackages"):
    if os.path.isdir(_p) and _p not in sys.path:
        sys.path.append(_p)

import numpy as np

import concourse.bass as bass
from concourse import bacc
import concourse.mybir as mybir
from concourse.tile import TileContext
from concourse.bass_utils import run_bass_kernel_spmd

F32 = mybir.dt.float32
F16 = mybir.dt.float16
F32R = mybir.dt.float32r
I32 = mybir.dt.int32
AF = mybir.ActivationFunctionType
MULT = mybir.AluOpType.mult
ADD = mybir.AluOpType.add

B, S, D = 2, 2048, 1024
H, NK, HD = 16, 21, 64
QS = 512
NCORES = 8
NPAIR = H // 2
NKB = S // 128
EBW = 2432
EUW = 2560
GROUPS = ((0, 112), (112, 112), (224, 112))


def build(debug=False):
    nc = bacc.Bacc("TRN2", target_bir_lowering=False, debug=False)

    xT_d = nc.dram_tensor("xT", [D, S], F16, kind="ExternalInput")
    xq_d = nc.dram_tensor("xq", [D, QS], F16, kind="ExternalInput")
    w_in_d = nc.dram_tensor("w_in", [D, 3 * D], F16, kind="ExternalInput")
    w_gate_d = nc.dram_tensor("w_gate", [D, 2 * D], F16, kind="ExternalInput")
    b_gate_d = nc.dram_tensor("b_gate", [1, 2 * D], F32R, kind="ExternalInput")
    eamp_d = nc.dram_tensor("eamp", [336, 16], F32R, kind="ExternalInput")
    offp_d = nc.dram_tensor("offp", [336, 1], F32, kind="ExternalInput")
    shp_d = nc.dram_tensor("shp", [336, 1], F32, kind="ExternalInput")
    rsh_d = nc.dram_tensor("rsh", [1, 1], F32, kind="ExternalInput")
    ones_d = nc.dram_tensor("ones", [1, 128], F32R, kind="ExternalInput")
    sel_d = nc.dram_tensor("sel", [16, 1024], F32R, kind="ExternalInput")

    out_d = nc.dram_tensor("out", [QS, D], F32, kind="ExternalOutput")
    eu_dram = nc.dram_tensor("eu_scratch", [H, EUW], F16)
    if debug:
        eu_dbg = nc.dram_tensor("eu_dbg", [H, EUW], F16, kind="ExternalOutput")
        qt_dbg = nc.dram_tensor("qt_dbg", [128, QS], F32R, kind="ExternalOutput")
        kp_dbg = nc.dram_tensor("kp_dbg", [128, S], F32R, kind="ExternalOutput")
        wt_dbg = nc.dram_tensor("wt_dbg", [128, 1024], F32R, kind="ExternalOutput")
        po_dbg = nc.dram_tensor("po_dbg", [65, 1024], F32, kind="ExternalOutput")
        v_dbg = nc.dram_tensor("v_dbg", [128, 260], F32R, kind="ExternalOutput")

    with TileContext(nc) as tc:
        with tc.tile_pool(name="gpool", bufs=1) as gpool:
          with tc.tile_pool(name="psum_m", bufs=3, space="PSUM") as psm, \
               tc.tile_pool(name="psum_o", bufs=2, space="PSUM") as pso:
            ones_sb = gpool.tile([1, 128], F32R, name="ones_sb")
            nc.sync.dma_start(ones_sb[:, :], ones_d[:, :])
            sel_sb = gpool.tile([16, 1024], F32R, name="sel_sb")
            nc.sync.dma_start(sel_sb[:, :], sel_d[:, :])
            # xq (dies after Q proj) shares tags with outT (written later)
            xq_sb = [gpool.tile([128, QS], F16, name=f"xq{d}", tag=f"xo{d}")
                     for d in range(8)]
            for d in range(8):
                nc.sync.dma_start(xq_sb[d][:, :], xq_d[128 * d:128 * d + 128, :])

            # ============ TISA tables ============
            with tc.tile_pool(name="upool", bufs=2) as upool:
                eu_ps_t = [psm.tile([128, 1024], F32, name=f"eups{i}", tag="pm")
                           for i in range(3)]
                eu_ps = [eu_ps_t[c // 2][0:16, 512 * (c % 2):512 * (c % 2) + 512]
                         for c in range(5)]
                # iota strips are identical for every amplitude group: hoist
                # them (15 -> 5 gpsimd ops; each costs ~1.2us serial)
                iofs = []
                for c in range(5):
                    t = upool.tile([112, 512], F32, name=f"iof{c}",
                                   tag=f"u8{c}", bufs=1)
                    nc.gpsimd.iota(t[:, :], pattern=[[1, 512]], base=512 * c,
                                   channel_multiplier=0,
                                   allow_small_or_imprecise_dtypes=True)
                    iofs.append(t)
                for gi, (g0, grows) in enumerate(GROUPS):
                    offg = upool.tile([112, 1], F32, name="offg", tag="u1")
                    nc.sync.dma_start(offg[:, :], offp_d[g0:g0 + 112, :])
                    rshg = upool.tile([112, 1], F32, name="rshg", tag="u2")
                    nc.sync.dma_start(rshg[:, :],
                                      bass.AP(rsh_d, 0, [[0, 112], [1, 1]]))
                    shg = upool.tile([112, 1], F32, name="shg", tag="u3")
                    nc.sync.dma_start(shg[:, :], shp_d[g0:g0 + 112, :])
                    negP = upool.tile([112, 1], F32, name="negP", tag="u4")
                    nc.vector.tensor_tensor(negP[:, :], offg[:, :], rshg[:, :], ADD)
                    nc.vector.tensor_scalar_mul(negP[:, :], negP[:, :], -1.0)
                    negsh = upool.tile([112, 1], F32, name="negsh", tag="u5")
                    nc.vector.tensor_scalar_mul(negsh[:, :], shg[:, :], -1.0)
                    Eg = upool.tile([112, 16], F32R, name="Eg", tag="u6")
                    nc.sync.dma_start(Eg[:, :], eamp_d[g0:g0 + 112, :])
                    for c in range(5):  # 512-wide strips of y
                        sqg = upool.tile([112, 512], F32, name="sqg", tag="u9")
                        nc.scalar.activation(sqg[:, :], iofs[c][:, :], AF.Square,
                                             bias=negP[:, 0:1])
                        Gg = upool.tile([112, 512], F32R, name="Gg", tag="u10")
                        nc.scalar.activation(Gg[:, :], sqg[:, :], AF.Exp,
                                             scale=negsh[:, 0:1])
                        nc.tensor.matmul(eu_ps[c], Eg[:, :], Gg[:, :],
                                         start=(gi == 0), stop=(gi == 2),
                                         skip_group_check=True)
                for c in range(5):
                    eu_sb = upool.tile([16, 512], F16, name="eu_sb", tag="u11")
                    nc.scalar.activation(eu_sb[:, :], eu_ps[c], AF.Exp)
                    nc.sync.dma_start(eu_dram[:, 512 * c:512 * c + 512],
                                      eu_sb[:, :])
                    if debug:
                        nc.sync.dma_start(eu_dbg[:, 512 * c:512 * c + 512],
                                          eu_sb[:, :])

            # ============ main span ============
            with tc.tile_pool(name="span", bufs=1) as span, \
                 tc.tile_pool(name="strm", bufs=2) as strm:

                xT_sb = [span.tile([128, S], F16, name=f"xT{d}")
                         for d in range(8)]
                for d in range(8):
                    nc.sync.dma_start(xT_sb[d][:, :],
                                      xT_d[128 * d:128 * d + 128, :])

                # ---- Q^T proj (upfront) ----
                qT_sb = [span.tile([128, QS], F16, name=f"qT{cb}")
                         for cb in range(8)]
                for quad in range(2):
                    pq = [psm.tile([128, 1024], F32, name=f"pq{quad}{t}",
                                   tag="pm") for t in range(2)]
                    tgt = [pq[0][:, 0:512], pq[0][:, 512:1024],
                           pq[1][:, 0:512], pq[1][:, 512:1024]]
                    for d in range(8):
                        wq = strm.tile([128, 512], F16, name="wq", tag="w",
                                       bufs=8)
                        nc.sync.dma_start(
                            wq[:, :],
                            w_in_d[128 * d:128 * d + 128,
                                   2 * D + 512 * quad:2 * D + 512 * quad + 512])
                        for t in range(4):
                            nc.tensor.matmul(
                                tgt[t], wq[:, 128 * t:128 * t + 128],
                                xq_sb[d][:, :], start=(d == 0), stop=(d == 7),
                                skip_group_check=True)
                    for t in range(4):
                        nc.vector.tensor_copy(qT_sb[4 * quad + t][:, :], tgt[t])

                # ---- attention with JIT K/V proj ----
                v_sb = [[span.tile([128, 260], F16, name=f"v{gg}_{kb}",
                                   tag=f"v{gg % 2}_{kb}")
                         for kb in range(NKB)] for gg in range(4)]
                kp_pool = [span.tile([128, S], F16, name=f"kp{i}")
                           for i in range(2)]
                outT_sb = [gpool.tile([128, QS], F16, name=f"outT{p}",
                                      tag=f"xo{p}") for p in range(NPAIR)]
                den_sb = span.tile([16, QS], F32, name="den_sb")

                def vproj_ops(g):
                    ops = []
                    wv = []

                    def load_wv():
                        for d in range(8):
                            t = strm.tile([128, 256], F16, name="wv", tag="w",
                                          bufs=8)
                            nc.sync.dma_start(
                                t[:, :],
                                w_in_d[128 * d:128 * d + 128,
                                       D + 256 * g:D + 256 * g + 256])
                            wv.append(t)
                    ops.append(load_wv)
                    for kq in range(4):
                        def mkv(kq):
                            def f():
                                # each 256-wide target bank-aligned: start=True
                                # clears the whole bank, so no two kb tiles may
                                # share a bank.
                                psv = [psm.tile([128, 1024], F32, name="psv",
                                                tag="pm") for _ in range(2)]
                                tg = [psv[t // 2][:, 512 * (t % 2):
                                                  512 * (t % 2) + 256]
                                      for t in range(4)]
                                for d in range(8):
                                    for t in range(4):
                                        kb = 4 * kq + t
                                        nc.tensor.matmul(
                                            tg[t],
                                            xT_sb[d][:, 128 * kb:128 * kb + 128],
                                            wv[d][:, :], start=(d == 0),
                                            stop=(d == 7), skip_group_check=True)
                                for t in range(4):
                                    kb = 4 * kq + t
                                    dst = v_sb[g][kb][:, :].rearrange(
                                        "p (j w) -> p j w", w=65)[:, :, 0:64]
                                    nc.scalar.activation(
                                        dst,
                                        tg[t].rearrange("p (j w) -> p j w", w=64),
                                        AF.Identity)
                                    oc = v_sb[g][kb][:, :].rearrange(
                                        "p (j w) -> p j w", w=65)[:, :, 64:65]
                                    nc.gpsimd.memset(oc, 1.0)
                            return f
                        ops.append(mkv(kq))
                    return ops

                def kproj_ops(p):
                    ops = []
                    kp = kp_pool[p % 2]
                    wk = []

                    def load_wk():
                        for d in range(8):
                            t = strm.tile([128, 128], F16, name="wk", tag="w",
                                          bufs=8)
                            nc.sync.dma_start(
                                t[:, :],
                                w_in_d[128 * d:128 * d + 128,
                                       128 * p:128 * p + 128])
                            wk.append(t)
                    ops.append(load_wk)
                    for half in range(2):
                        def mkk(half):
                            def f():
                                psk = psm.tile([128, 1024], F32, name="psk",
                                               tag="pm")
                                for d in range(8):
                                    for t in range(2):
                                        nc.tensor.matmul(
                                            psk[:, 512 * t:512 * t + 512],
                                            wk[d][:, :],
                                            xT_sb[d][:, 1024 * half + 512 * t:
                                                     1024 * half + 512 * t + 512],
                                            start=(d == 0), stop=(d == 7),
                                            skip_group_check=True)
                                nc.vector.tensor_copy(
                                    kp[:, 1024 * half:1024 * half + 1024],
                                    psk[:, :])
                            return f
                        ops.append(mkk(half))
                    return ops

                # prologue: V group 0, K pair 0
                for op in vproj_ops(0):
                    op()
                for op in kproj_ops(0):
                    op()

                # prefetch gate weights during attention: loading them when
                # the gate pool opens (after the span pools close) exposes
                # ~4MB of DMA latency as a tensor-idle dip before the gate.
                wg_sb = [[gpool.tile([128, 1024], F16, name=f"wg{ph}{d}")
                          for d in range(8)] for ph in range(2)]
                for ph in range(2):
                    for d in range(8):
                        nc.sync.dma_start(
                            wg_sb[ph][d][:, :],
                            bass.AP(w_gate_d, 128 * d * 2 * D + 512 * ph,
                                    [[2 * D, 128], [1024, 2], [1, 512]]))

                if debug:
                    nc.sync.dma_start(qt_dbg[:, :], qT_sb[0][:, :])
                    nc.sync.dma_start(kp_dbg[:, :], kp_pool[0][:, :])
                    nc.sync.dma_start(v_dbg[:, :], v_sb[0][0][:, :])
                for p in range(NPAIR):
                    hA = 2 * p
                    g, j0 = p // 2, 2 * (p % 2)
                    kp = kp_pool[p % 2]
                    eb = strm.tile([128, 2 * EBW], F16, name="eb", tag="eb",
                                   bufs=3)
                    for hi in range(2):
                        # Queries run REVERSED (host feeds xq columns
                        # backwards), so the bias factor for score tile kb at
                        # [p, j] is eu[h, 128*kb + p + j]: load the diagonal
                        # table as eb[p, x'] = eu[h, p + x'] -- all strides +1
                        # and contiguous (a -1 stride here costs one 2-byte
                        # DMA descriptor per element: 5M packets, 5.6 ms).
                        nc.sync.dma_start(
                            eb[:, EBW * hi:EBW * hi + EBW],
                            bass.AP(eu_dram, (hA + hi) * EUW,
                                    [[1, 128], [1, EBW]]))
                    pend = []
                    if p + 1 < NPAIR:
                        if (p + 1) % 2 == 0:
                            pend += vproj_ops((p + 1) // 2)
                        pend += kproj_ops(p + 1)
                    slots = [[] for _ in range(NKB)]
                    for i, op in enumerate(pend):
                        slots[min(2 + i * 2, NKB - 1)].append(op)

                    po = pso.tile([65, 512], F32, name="po", tag="po")
                    po2 = pso.tile([65, 512], F32, name="po2", tag="po")
                    for kb in range(NKB):
                        psc = psm.tile([128, 1024], F32, name="psc", tag="pm")
                        nc.tensor.matmul(psc[:, 0:512],
                                         kp[0:64, 128 * kb:128 * kb + 128],
                                         qT_sb[p][0:64, :], start=True,
                                         stop=True)
                        nc.tensor.matmul(psc[:, 512:1024],
                                         kp[64:128, 128 * kb:128 * kb + 128],
                                         qT_sb[p][64:128, :], start=True,
                                         stop=True)
                        et = strm.tile([128, 1024], F16, name="et", tag="et",
                                       bufs=3)
                        nc.scalar.activation(et[:, :], psc[:, :], AF.Exp,
                                             scale=0.125)
                        wt = strm.tile([128, 1024], F16, name="wt", tag="wt",
                                       bufs=3)
                        delta = 128 * kb
                        ebv = eb[:, :].rearrange("p (i x) -> p i x", i=2)[
                            :, :, delta:delta + 512]
                        nc.vector.tensor_tensor(
                            wt[:, :].rearrange("p (i q) -> p i q", i=2),
                            et[:, :].rearrange("p (i q) -> p i q", i=2),
                            ebv, MULT)
                        if debug and p == 0 and kb == 5:
                            nc.sync.dma_start(wt_dbg[:, :], wt[:, :])
                        nc.tensor.matmul(
                            po[:, :], v_sb[g][kb][:, 65 * j0:65 * j0 + 65],
                            wt[:, 0:512], start=(kb == 0),
                            stop=(kb == NKB - 1), skip_group_check=True)
                        nc.tensor.matmul(
                            po2[:, :],
                            v_sb[g][kb][:, 65 * (j0 + 1):65 * (j0 + 1) + 65],
                            wt[:, 512:1024], start=(kb == 0),
                            stop=(kb == NKB - 1), skip_group_check=True)
                        for op in slots[kb]:
                            op()
                    for hi, pot in enumerate((po, po2)):
                        if debug and p == 0:
                            pod = strm.tile([65, 512], F32, name="pod", tag="pod")
                            nc.scalar.activation(pod[:, :], pot[:, :], AF.Identity)
                            nc.sync.dma_start(
                                po_dbg[:, 512 * hi:512 * hi + 512], pod[:, :])
                        # stash unnormalized attn + its denominator row; the
                        # reciprocal runs ONCE batched over [16, 512] after the
                        # pair loop (16 single-partition reciprocals = 64us of
                        # 8-cycle/elem DVE divide on one lane).
                        # engines need 32-aligned base partitions, so stage the
                        # denominator row at partition 0 and DMA it into its
                        # den_sb partition (DMA places partitions freely).
                        dstg = strm.tile([1, 512], F32, name="dstg", tag="dstg")
                        nc.scalar.activation(dstg[:, :], pot[64:65, :],
                                             AF.Identity)
                        nc.sync.dma_start(
                            den_sb[2 * p + hi:2 * p + hi + 1, :], dstg[:, :])
                        nc.vector.tensor_copy(
                            outT_sb[p][64 * hi:64 * hi + 64, :], pot[0:64, :])

                # ---- batched softmax normalization ----
                rden = span.tile([16, QS], F32R, name="rden")
                with nc.allow_low_precision(
                        reason="f32r reciprocal: 1.2e-4 rel is fine"):
                    nc.vector.reciprocal(rden[:, :], den_sb[:, :])
                for p in range(NPAIR):
                    pbt = psm.tile([128, 1024], F32, name="pbt", tag="pm")
                    pb = pbt[:, 0:512]
                    # sel block p: pb[c, q] = rden[2p + c//64, q]
                    nc.tensor.matmul(pb, sel_sb[:, 128 * p:128 * p + 128],
                                     rden[:, :], start=True, stop=True)
                    nc.vector.tensor_tensor(
                        outT_sb[p][:, :], outT_sb[p][:, :], pb, MULT)

          # ============ gate + GLU ============
          with tc.tile_pool(name="gate", bufs=2) as gp, \
               tc.tile_pool(name="psum_g", bufs=1, space="PSUM") as psg:
              bg_sb = gp.tile([1, 2 * D], F32R, name="bg_sb", bufs=1)
              nc.sync.dma_start(bg_sb[:, :], b_gate_d[:, :])
              for ph in range(2):
                  pgt = [psg.tile([128, 512], F32, name=f"pg{ph}{i}",
                                  tag=f"pg{i}") for i in range(8)]
                  for d in range(8):
                      wg = wg_sb[ph][d]
                      for qb in range(4):
                          for ci in range(2):
                              nc.tensor.matmul(
                                  pgt[2 * qb + ci],
                                  outT_sb[d][:, 128 * qb:128 * qb + 128],
                                  wg[:, 512 * ci:512 * ci + 512],
                                  start=(d == 0), stop=False,
                                  skip_group_check=True)
                  for qb in range(4):
                      for ci in range(2):
                          nc.tensor.matmul(
                              pgt[2 * qb + ci], ones_sb[:, :],
                              bg_sb[:, 1024 * ci + 512 * ph:
                                    1024 * ci + 512 * ph + 512],
                              start=False, stop=True, skip_group_check=True)
                  for qb in range(4):
                      sg = gp.tile([128, 512], F32, name="sg", tag="sg")
                      nc.scalar.activation(sg[:, :], pgt[2 * qb + 1], AF.Sigmoid)
                      res = gp.tile([128, 512], F32, name="res", tag="res")
                      nc.vector.tensor_tensor(res[:, :], pgt[2 * qb], sg[:, :],
                                              MULT)
                      nc.sync.dma_start(
                          out_d[128 * qb:128 * qb + 128,
                                512 * ph:512 * ph + 512],
                          res[:, :])

    nc.finalize()
    return nc


_NC_CACHE = None
_LAST_IN_MAPS = None


def _get_nc():
    global _NC_CACHE
    if _NC_CACHE is None:
        _NC_CACHE = build()
    return _NC_CACHE


def kernel(x, w_in, w_gate, b_gate, amplitudes, sharpness, offsets):
    x = np.ascontiguousarray(x, dtype=np.float32)
    w_in16 = np.ascontiguousarray(w_in, dtype=np.float16)
    w_gate16 = np.ascontiguousarray(w_gate, dtype=np.float16)
    b_gate = np.ascontiguousarray(b_gate, dtype=np.float32).reshape(1, 2 * D)
    amplitudes = np.asarray(amplitudes, dtype=np.float32)
    sharpness = np.asarray(sharpness, dtype=np.float32)
    offsets = np.asarray(offsets, dtype=np.float32)

    eamp = np.zeros((H * NK, 16), np.float32)
    eamp[np.arange(H * NK), np.arange(H * NK) // NK] = amplitudes.reshape(-1)
    offp = offsets.reshape(H * NK, 1)
    shp = sharpness.reshape(H * NK, 1)
    ones = np.ones((1, 128), np.float32)
    # sel[r, 128p + c] = 1 iff r == 2p + c//64 (head selector used to
    # broadcast the batched softmax reciprocals to 128 output rows per pair)
    sel = np.zeros((16, 1024), np.float32)
    for p_ in range(8):
        sel[2 * p_, 128 * p_:128 * p_ + 64] = 1.0
        sel[2 * p_ + 1, 128 * p_ + 64:128 * p_ + 128] = 1.0

    in_maps = []
    for c in range(NCORES):
        b, r = c // 4, c % 4
        xT = np.ascontiguousarray(x[b].T, dtype=np.float16)
        # query columns fed in REVERSED order so the TISA bias slice per
        # k-block is an ascending (contiguous-DMA) slice of the eu table;
        # the output rows are un-reversed after the run.
        xq = np.ascontiguousarray(x[b, QS * r:QS * r + QS, :].T[:, ::-1],
                                  dtype=np.float16)
        rsh = np.array([[511.0 + 512.0 * r]], np.float32)
        in_maps.append({
            "xT": xT, "xq": xq, "w_in": w_in16, "w_gate": w_gate16,
            "b_gate": b_gate, "eamp": eamp, "offp": offp, "shp": shp,
            "rsh": rsh, "ones": ones, "sel": sel,
        })

    global _LAST_IN_MAPS
    _LAST_IN_MAPS = in_maps
    nc = _get_nc()
    r_ = run_bass_kernel_spmd(nc, in_maps, core_ids=list(range(NCORES)))
    out = np.empty((B, S, D), np.float32)
    for c in range(NCORES):
        b, r = c // 4, c % 4
        out[b, QS * r:QS * r + QS, :] = r_.results[c]["out"][::-1, :]
    return out



# revision 64
# speedup vs baseline: 1.0494x; 1.0100x over previous
"""Trainium2 Bass kernel for nn_GatedAttn (gated attention with TISA bias).

Takes FULL inputs, returns FULL output. 8 NeuronCores, sharded as
(batch b = core//4) x (query-row slice r = core%4, 512 rows each); each core
runs the whole pipeline for its 512 query rows (K^T/V projections are
recomputed per core -- an AllGather variant that shares them across the
batch's 4 cores was measured SLOWER: the DRAM-DRAM collective exposes
~130us of latency that the saved PE time cannot cover).

Queries are processed in REVERSED order (host feeds xq columns backwards and
un-reverses output rows) so the per-k-block TISA bias factor is an ascending
contiguous slice of the eu table -- a descending slice would cost one 2-byte
DMA descriptor per element (5M packets = 5.6 ms, the original bottleneck).

Per-core pipeline (all projection/attention matmuls in fp16 operands with
fp32 PSUM accumulation; rel err ~1.4e-3 vs the 2e-2 gate):
  u-tables:  u[h,y] = sum_k amp*exp(-sh*(y-(511+512r+off))^2) via ACT
             Square/Exp + an amplitude-selector matmul; eu = exp(u) (fp16) to
             DRAM; per head-pair load EB[p,x'] = eu[h, p+x'] (all strides +1).
  proj:      Q^T upfront from xq (the core's 512 columns of x^T); K^T and
             V computed just-in-time per head pair / 4-head group, with the
             proj matmuls software-pipelined into the previous pair's
             attention loop. V gets a ones column per head (V_aug, 65-wide)
             so the AV matmul also produces softmax row sums for free.
  attention: scores^T tiles (k_pos x q) via QK matmuls (contraction hd=64,
             head pairs at base partitions 0/64). Softmax without
             max-subtraction (|score| <= ~8.1): ACT exp (PSUM f32 -> SBUF
             fp16), DVE 2x-mode fp16 multiply with the EB table, fp16 AV
             matmuls; attn^T accumulates over 16 k-blocks in PSUM, row 64 =
             denominators. Denominator rows are staged to a [16,512] tile;
             ONE batched DVE reciprocal (a [1,512] reciprocal is 4.3us of
             single-lane 8-cycle divide), then per pair one selector matmul
             broadcasts both heads' reciprocals to 128 rows + one DVE mult.
  gate:      (512 q x 2048) = out^T @ w_gate + b_gate (K=1 ones matmul),
             a * sigmoid(g) -> (512, 1024) output slice.

fp32r/fp16 PSUM-accumulation hazard: accumulating matmuls into a bank need
>=3 intervening matmuls -> all accumulation loops rotate >=4 bank targets.
"""

import sys
import os

for _p in ("/opt/trn_rl_repo", "/opt/pypackages"):
    if os.path.isdir(_p) and _p not in sys.path:
        sys.path.append(_p)

import numpy as np

import concourse.bass as bass
from concourse import bacc
import concourse.mybir as mybir
from concourse.tile import TileContext
from concourse.bass_utils import run_bass_kernel_spmd

F32 = mybir.dt.float32
F16 = mybir.dt.float16
F32R = mybir.dt.float32r
I32 = mybir.dt.int32
AF = mybir.ActivationFunctionType
MULT = mybir.AluOpType.mult
ADD = mybir.AluOpType.add

B, S, D = 2, 2048, 1024
H, NK, HD = 16, 21, 64
QS = 512
NCORES = 8
NPAIR = H // 2
NKB = S // 128
EBW = 2432
EUW = 2560
GROUPS = ((0, 112), (112, 112), (224, 112))


def build(debug=False):
    nc = bacc.Bacc("TRN2", target_bir_lowering=False, debug=False)

    xT_d = nc.dram_tensor("xT", [D, S], F16, kind="ExternalInput")
    xq_d = nc.dram_tensor("xq", [D, QS], F16, kind="ExternalInput")
    w_in_d = nc.dram_tensor("w_in", [D, 3 * D], F16, kind="ExternalInput")
    w_gate_d = nc.dram_tensor("w_gate", [D, 2 * D], F16, kind="ExternalInput")
    b_gate_d = nc.dram_tensor("b_gate", [1, 2 * D], F32R, kind="ExternalInput")
    eamp_d = nc.dram_tensor("eamp", [336, 16], F32R, kind="ExternalInput")
    offp_d = nc.dram_tensor("offp", [336, 1], F32, kind="ExternalInput")
    shp_d = nc.dram_tensor("shp", [336, 1], F32, kind="ExternalInput")
    rsh_d = nc.dram_tensor("rsh", [1, 1], F32, kind="ExternalInput")
    ones_d = nc.dram_tensor("ones", [1, 128], F32R, kind="ExternalInput")
    sel_d = nc.dram_tensor("sel", [16, 1024], F32R, kind="ExternalInput")

    out_d = nc.dram_tensor("out", [QS, D], F32, kind="ExternalOutput")
    eu_dram = nc.dram_tensor("eu_scratch", [H, EUW], F16)
    if debug:
        eu_dbg = nc.dram_tensor("eu_dbg", [H, EUW], F16, kind="ExternalOutput")
        qt_dbg = nc.dram_tensor("qt_dbg", [128, QS], F32R, kind="ExternalOutput")
        kp_dbg = nc.dram_tensor("kp_dbg", [128, S], F32R, kind="ExternalOutput")
        wt_dbg = nc.dram_tensor("wt_dbg", [128, 1024], F32R, kind="ExternalOutput")
        po_dbg = nc.dram_tensor("po_dbg", [65, 1024], F32, kind="ExternalOutput")
        v_dbg = nc.dram_tensor("v_dbg", [128, 260], F32R, kind="ExternalOutput")

    with TileContext(nc) as tc:
        with tc.tile_pool(name="gpool", bufs=1) as gpool:
          with tc.tile_pool(name="psum_m", bufs=3, space="PSUM") as psm, \
               tc.tile_pool(name="psum_o", bufs=2, space="PSUM") as pso:
            ones_sb = gpool.tile([1, 128], F32R, name="ones_sb")
            nc.sync.dma_start(ones_sb[:, :], ones_d[:, :])
            sel_sb = gpool.tile([16, 1024], F32R, name="sel_sb")
            nc.sync.dma_start(sel_sb[:, :], sel_d[:, :])
            # xq (dies after Q proj) shares tags with outT (written later)
            xq_sb = [gpool.tile([128, QS], F16, name=f"xq{d}", tag=f"xo{d}")
                     for d in range(8)]
            for d in range(8):
                nc.sync.dma_start(xq_sb[d][:, :], xq_d[128 * d:128 * d + 128, :])

            # ============ TISA tables ============
            with tc.tile_pool(name="upool", bufs=2) as upool:
                eu_ps_t = [psm.tile([128, 1024], F32, name=f"eups{i}", tag="pm")
                           for i in range(3)]
                eu_ps = [eu_ps_t[c // 2][0:16, 512 * (c % 2):512 * (c % 2) + 512]
                         for c in range(5)]
                # iota strips are identical for every amplitude group: hoist
                # them (15 -> 5 gpsimd ops; each costs ~1.2us serial)
                iofs = []
                for c in range(5):
                    t = upool.tile([112, 512], F32, name=f"iof{c}",
                                   tag=f"u8{c}", bufs=1)
                    nc.gpsimd.iota(t[:, :], pattern=[[1, 512]], base=512 * c,
                                   channel_multiplier=0,
                                   allow_small_or_imprecise_dtypes=True)
                    iofs.append(t)
                for gi, (g0, grows) in enumerate(GROUPS):
                    offg = upool.tile([112, 1], F32, name="offg", tag="u1")
                    nc.sync.dma_start(offg[:, :], offp_d[g0:g0 + 112, :])
                    rshg = upool.tile([112, 1], F32, name="rshg", tag="u2")
                    nc.sync.dma_start(rshg[:, :],
                                      bass.AP(rsh_d, 0, [[0, 112], [1, 1]]))
                    shg = upool.tile([112, 1], F32, name="shg", tag="u3")
                    nc.sync.dma_start(shg[:, :], shp_d[g0:g0 + 112, :])
                    negP = upool.tile([112, 1], F32, name="negP", tag="u4")
                    nc.vector.tensor_tensor(negP[:, :], offg[:, :], rshg[:, :], ADD)
                    nc.vector.tensor_scalar_mul(negP[:, :], negP[:, :], -1.0)
                    negsh = upool.tile([112, 1], F32, name="negsh", tag="u5")
                    nc.vector.tensor_scalar_mul(negsh[:, :], shg[:, :], -1.0)
                    Eg = upool.tile([112, 16], F32R, name="Eg", tag="u6")
                    nc.sync.dma_start(Eg[:, :], eamp_d[g0:g0 + 112, :])
                    for c in range(5):  # 512-wide strips of y
                        sqg = upool.tile([112, 512], F32, name="sqg", tag="u9")
                        nc.scalar.activation(sqg[:, :], iofs[c][:, :], AF.Square,
                                             bias=negP[:, 0:1])
                        Gg = upool.tile([112, 512], F32R, name="Gg", tag="u10")
                        nc.scalar.activation(Gg[:, :], sqg[:, :], AF.Exp,
                                             scale=negsh[:, 0:1])
                        nc.tensor.matmul(eu_ps[c], Eg[:, :], Gg[:, :],
                                         start=(gi == 0), stop=(gi == 2),
                                         skip_group_check=True)
                for c in range(5):
                    eu_sb = upool.tile([16, 512], F16, name="eu_sb", tag="u11")
                    nc.scalar.activation(eu_sb[:, :], eu_ps[c], AF.Exp)
                    nc.sync.dma_start(eu_dram[:, 512 * c:512 * c + 512],
                                      eu_sb[:, :])
                    if debug:
                        nc.sync.dma_start(eu_dbg[:, 512 * c:512 * c + 512],
                                          eu_sb[:, :])

            # ============ main span ============
            with tc.tile_pool(name="span", bufs=1) as span, \
                 tc.tile_pool(name="strm", bufs=2) as strm:

                xT_sb = [span.tile([128, S], F16, name=f"xT{d}")
                         for d in range(8)]
                for d in range(8):
                    nc.sync.dma_start(xT_sb[d][:, :],
                                      xT_d[128 * d:128 * d + 128, :])

                # ---- Q^T proj (upfront) ----
                qT_sb = [span.tile([128, QS], F16, name=f"qT{cb}")
                         for cb in range(8)]
                for quad in range(2):
                    pq = [psm.tile([128, 1024], F32, name=f"pq{quad}{t}",
                                   tag="pm") for t in range(2)]
                    tgt = [pq[0][:, 0:512], pq[0][:, 512:1024],
                           pq[1][:, 0:512], pq[1][:, 512:1024]]
                    for d in range(8):
                        wq = strm.tile([128, 512], F16, name="wq", tag="w",
                                       bufs=16)
                        nc.sync.dma_start(
                            wq[:, :],
                            w_in_d[128 * d:128 * d + 128,
                                   2 * D + 512 * quad:2 * D + 512 * quad + 512])
                        for t in range(4):
                            nc.tensor.matmul(
                                tgt[t], wq[:, 128 * t:128 * t + 128],
                                xq_sb[d][:, :], start=(d == 0), stop=(d == 7),
                                skip_group_check=True)
                    for t in range(4):
                        nc.vector.tensor_copy(qT_sb[4 * quad + t][:, :], tgt[t])

                # ---- attention with JIT K/V proj ----
                v_sb = [[span.tile([128, 260], F16, name=f"v{gg}_{kb}",
                                   tag=f"v{gg % 2}_{kb}")
                         for kb in range(NKB)] for gg in range(4)]
                kp_pool = [span.tile([128, S], F16, name=f"kp{i}")
                           for i in range(2)]
                outT_sb = [gpool.tile([128, QS], F16, name=f"outT{p}",
                                      tag=f"xo{p}") for p in range(NPAIR)]
                den_sb = span.tile([16, QS], F32, name="den_sb")

                def vproj_ops(g):
                    ops = []
                    wv = []

                    def load_wv():
                        for d in range(8):
                            t = strm.tile([128, 256], F16, name="wv", tag="w",
                                          bufs=16)
                            nc.sync.dma_start(
                                t[:, :],
                                w_in_d[128 * d:128 * d + 128,
                                       D + 256 * g:D + 256 * g + 256])
                            wv.append(t)
                    ops.append(load_wv)
                    for kq in range(4):
                        def mkv(kq):
                            def f():
                                # each 256-wide target bank-aligned: start=True
                                # clears the whole bank, so no two kb tiles may
                                # share a bank.
                                psv = [psm.tile([128, 1024], F32, name="psv",
                                                tag="pm") for _ in range(2)]
                                tg = [psv[t // 2][:, 512 * (t % 2):
                                                  512 * (t % 2) + 256]
                                      for t in range(4)]
                                for d in range(8):
                                    for t in range(4):
                                        kb = 4 * kq + t
                                        nc.tensor.matmul(
                                            tg[t],
                                            xT_sb[d][:, 128 * kb:128 * kb + 128],
                                            wv[d][:, :], start=(d == 0),
                                            stop=(d == 7), skip_group_check=True)
                                for t in range(4):
                                    kb = 4 * kq + t
                                    dst = v_sb[g][kb][:, :].rearrange(
                                        "p (j w) -> p j w", w=65)[:, :, 0:64]
                                    nc.scalar.activation(
                                        dst,
                                        tg[t].rearrange("p (j w) -> p j w", w=64),
                                        AF.Identity)
                                    oc = v_sb[g][kb][:, :].rearrange(
                                        "p (j w) -> p j w", w=65)[:, :, 64:65]
                                    nc.gpsimd.memset(oc, 1.0)
                            return f
                        ops.append(mkv(kq))
                    return ops

                def kproj_ops(p):
                    ops = []
                    kp = kp_pool[p % 2]
                    wk = []

                    def load_wk():
                        for d in range(8):
                            t = strm.tile([128, 128], F16, name="wk", tag="w",
                                          bufs=16)
                            nc.sync.dma_start(
                                t[:, :],
                                w_in_d[128 * d:128 * d + 128,
                                       128 * p:128 * p + 128])
                            wk.append(t)
                    ops.append(load_wk)
                    for half in range(2):
                        def mkk(half):
                            def f():
                                psk = psm.tile([128, 1024], F32, name="psk",
                                               tag="pm")
                                for d in range(8):
                                    for t in range(2):
                                        nc.tensor.matmul(
                                            psk[:, 512 * t:512 * t + 512],
                                            wk[d][:, :],
                                            xT_sb[d][:, 1024 * half + 512 * t:
                                                     1024 * half + 512 * t + 512],
                                            start=(d == 0), stop=(d == 7),
                                            skip_group_check=True)
                                nc.vector.tensor_copy(
                                    kp[:, 1024 * half:1024 * half + 1024],
                                    psk[:, :])
                            return f
                        ops.append(mkk(half))
                    return ops

                # prologue: V group 0, K pair 0
                for op in vproj_ops(0):
                    op()
                for op in kproj_ops(0):
                    op()

                # prefetch gate weights during attention: loading them when
                # the gate pool opens (after the span pools close) exposes
                # ~4MB of DMA latency as a tensor-idle dip before the gate.
                wg_sb = [[gpool.tile([128, 1024], F16, name=f"wg{ph}{d}")
                          for d in range(8)] for ph in range(2)]
                for ph in range(2):
                    for d in range(8):
                        nc.sync.dma_start(
                            wg_sb[ph][d][:, :],
                            bass.AP(w_gate_d, 128 * d * 2 * D + 512 * ph,
                                    [[2 * D, 128], [1024, 2], [1, 512]]))

                if debug:
                    nc.sync.dma_start(qt_dbg[:, :], qT_sb[0][:, :])
                    nc.sync.dma_start(kp_dbg[:, :], kp_pool[0][:, :])
                    nc.sync.dma_start(v_dbg[:, :], v_sb[0][0][:, :])
                for p in range(NPAIR):
                    hA = 2 * p
                    g, j0 = p // 2, 2 * (p % 2)
                    kp = kp_pool[p % 2]
                    eb = strm.tile([128, 2 * EBW], F16, name="eb", tag="eb",
                                   bufs=3)
                    for hi in range(2):
                        # Queries run REVERSED (host feeds xq columns
                        # backwards), so the bias factor for score tile kb at
                        # [p, j] is eu[h, 128*kb + p + j]: load the diagonal
                        # table as eb[p, x'] = eu[h, p + x'] -- all strides +1
                        # and contiguous (a -1 stride here costs one 2-byte
                        # DMA descriptor per element: 5M packets, 5.6 ms).
                        nc.sync.dma_start(
                            eb[:, EBW * hi:EBW * hi + EBW],
                            bass.AP(eu_dram, (hA + hi) * EUW,
                                    [[1, 128], [1, EBW]]))
                    pend = []
                    if p + 1 < NPAIR:
                        if (p + 1) % 2 == 0:
                            pend += vproj_ops((p + 1) // 2)
                        pend += kproj_ops(p + 1)
                    slots = [[] for _ in range(NKB)]
                    for i, op in enumerate(pend):
                        slots[min(2 + i * 2, NKB - 1)].append(op)

                    po = pso.tile([65, 512], F32, name="po", tag="po")
                    po2 = pso.tile([65, 512], F32, name="po2", tag="po")
                    for kb in range(NKB):
                        psc = psm.tile([128, 1024], F32, name="psc", tag="pm")
                        nc.tensor.matmul(psc[:, 0:512],
                                         kp[0:64, 128 * kb:128 * kb + 128],
                                         qT_sb[p][0:64, :], start=True,
                                         stop=True)
                        nc.tensor.matmul(psc[:, 512:1024],
                                         kp[64:128, 128 * kb:128 * kb + 128],
                                         qT_sb[p][64:128, :], start=True,
                                         stop=True)
                        et = strm.tile([128, 1024], F16, name="et", tag="et",
                                       bufs=3)
                        nc.scalar.activation(et[:, :], psc[:, :], AF.Exp,
                                             scale=0.125)
                        wt = strm.tile([128, 1024], F16, name="wt", tag="wt",
                                       bufs=3)
                        delta = 128 * kb
                        ebv = eb[:, :].rearrange("p (i x) -> p i x", i=2)[
                            :, :, delta:delta + 512]
                        nc.vector.tensor_tensor(
                            wt[:, :].rearrange("p (i q) -> p i q", i=2),
                            et[:, :].rearrange("p (i q) -> p i q", i=2),
                            ebv, MULT)
                        if debug and p == 0 and kb == 5:
                            nc.sync.dma_start(wt_dbg[:, :], wt[:, :])
                        nc.tensor.matmul(
                            po[:, :], v_sb[g][kb][:, 65 * j0:65 * j0 + 65],
                            wt[:, 0:512], start=(kb == 0),
                            stop=(kb == NKB - 1), skip_group_check=True)
                        nc.tensor.matmul(
                            po2[:, :],
                            v_sb[g][kb][:, 65 * (j0 + 1):65 * (j0 + 1) + 65],
                            wt[:, 512:1024], start=(kb == 0),
                            stop=(kb == NKB - 1), skip_group_check=True)
                        for op in slots[kb]:
                            op()
                    for hi, pot in enumerate((po, po2)):
                        if debug and p == 0:
                            pod = strm.tile([65, 512], F32, name="pod", tag="pod")
                            nc.scalar.activation(pod[:, :], pot[:, :], AF.Identity)
                            nc.sync.dma_start(
                                po_dbg[:, 512 * hi:512 * hi + 512], pod[:, :])
                        # stash unnormalized attn + its denominator row; the
                        # reciprocal runs ONCE batched over [16, 512] after the
                        # pair loop (16 single-partition reciprocals = 64us of
                        # 8-cycle/elem DVE divide on one lane).
                        # engines need 32-aligned base partitions, so stage the
                        # denominator row at partition 0 and DMA it into its
                        # den_sb partition (DMA places partitions freely).
                        dstg = strm.tile([1, 512], F32, name="dstg", tag="dstg")
                        nc.scalar.activation(dstg[:, :], pot[64:65, :],
                                             AF.Identity)
                        nc.sync.dma_start(
                            den_sb[2 * p + hi:2 * p + hi + 1, :], dstg[:, :])
                        nc.vector.tensor_copy(
                            outT_sb[p][64 * hi:64 * hi + 64, :], pot[0:64, :])

                # ---- batched softmax normalization ----
                rden = span.tile([16, QS], F32R, name="rden")
                with nc.allow_low_precision(
                        reason="f32r reciprocal: 1.2e-4 rel is fine"):
                    nc.vector.reciprocal(rden[:, :], den_sb[:, :])
                for p in range(NPAIR):
                    pbt = psm.tile([128, 1024], F32, name="pbt", tag="pm")
                    pb = pbt[:, 0:512]
                    # sel block p: pb[c, q] = rden[2p + c//64, q]
                    nc.tensor.matmul(pb, sel_sb[:, 128 * p:128 * p + 128],
                                     rden[:, :], start=True, stop=True)
                    nc.vector.tensor_tensor(
                        outT_sb[p][:, :], outT_sb[p][:, :], pb, MULT)

          # ============ gate + GLU ============
          with tc.tile_pool(name="gate", bufs=2) as gp, \
               tc.tile_pool(name="psum_g", bufs=1, space="PSUM") as psg:
              bg_sb = gp.tile([1, 2 * D], F32R, name="bg_sb", bufs=1)
              nc.sync.dma_start(bg_sb[:, :], b_gate_d[:, :])
              for ph in range(2):
                  pgt = [psg.tile([128, 512], F32, name=f"pg{ph}{i}",
                                  tag=f"pg{i}") for i in range(8)]
                  for d in range(8):
                      wg = wg_sb[ph][d]
                      for qb in range(4):
                          for ci in range(2):
                              nc.tensor.matmul(
                                  pgt[2 * qb + ci],
                                  outT_sb[d][:, 128 * qb:128 * qb + 128],
                                  wg[:, 512 * ci:512 * ci + 512],
                                  start=(d == 0), stop=False,
                                  skip_group_check=True)
                  for qb in range(4):
                      for ci in range(2):
                          nc.tensor.matmul(
                              pgt[2 * qb + ci], ones_sb[:, :],
                              bg_sb[:, 1024 * ci + 512 * ph:
                                    1024 * ci + 512 * ph + 512],
                              start=False, stop=True, skip_group_check=True)
                  for qb in range(4):
                      sg = gp.tile([128, 512], F32, name="sg", tag="sg")
                      nc.scalar.activation(sg[:, :], pgt[2 * qb + 1], AF.Sigmoid)
                      res = gp.tile([128, 512], F32, name="res", tag="res")
                      nc.vector.tensor_tensor(res[:, :], pgt[2 * qb], sg[:, :],
                                              MULT)
                      nc.sync.dma_start(
                          out_d[128 * qb:128 * qb + 128,
                                512 * ph:512 * ph + 512],
                          res[:, :])

    nc.finalize()
    return nc


_NC_CACHE = None
_LAST_IN_MAPS = None


def _get_nc():
    global _NC_CACHE
    if _NC_CACHE is None:
        _NC_CACHE = build()
    return _NC_CACHE


def kernel(x, w_in, w_gate, b_gate, amplitudes, sharpness, offsets):
    x = np.ascontiguousarray(x, dtype=np.float32)
    w_in16 = np.ascontiguousarray(w_in, dtype=np.float16)
    w_gate16 = np.ascontiguousarray(w_gate, dtype=np.float16)
    b_gate = np.ascontiguousarray(b_gate, dtype=np.float32).reshape(1, 2 * D)
    amplitudes = np.asarray(amplitudes, dtype=np.float32)
    sharpness = np.asarray(sharpness, dtype=np.float32)
    offsets = np.asarray(offsets, dtype=np.float32)

    eamp = np.zeros((H * NK, 16), np.float32)
    eamp[np.arange(H * NK), np.arange(H * NK) // NK] = amplitudes.reshape(-1)
    offp = offsets.reshape(H * NK, 1)
    shp = sharpness.reshape(H * NK, 1)
    ones = np.ones((1, 128), np.float32)
    # sel[r, 128p + c] = 1 iff r == 2p + c//64 (head selector used to
    # broadcast the batched softmax reciprocals to 128 output rows per pair)
    sel = np.zeros((16, 1024), np.float32)
    for p_ in range(8):
        sel[2 * p_, 128 * p_:128 * p_ + 64] = 1.0
        sel[2 * p_ + 1, 128 * p_ + 64:128 * p_ + 128] = 1.0

    in_maps = []
    for c in range(NCORES):
        b, r = c // 4, c % 4
        xT = np.ascontiguousarray(x[b].T, dtype=np.float16)
        # query columns fed in REVERSED order so the TISA bias slice per
        # k-block is an ascending (contiguous-DMA) slice of the eu table;
        # the output rows are un-reversed after the run.
        xq = np.ascontiguousarray(x[b, QS * r:QS * r + QS, :].T[:, ::-1],
                                  dtype=np.float16)
        rsh = np.array([[511.0 + 512.0 * r]], np.float32)
        in_maps.append({
            "xT": xT, "xq": xq, "w_in": w_in16, "w_gate": w_gate16,
            "b_gate": b_gate, "eamp": eamp, "offp": offp, "shp": shp,
            "rsh": rsh, "ones": ones, "sel": sel,
        })

    global _LAST_IN_MAPS
    _LAST_IN_MAPS = in_maps
    nc = _get_nc()
    r_ = run_bass_kernel_spmd(nc, in_maps, core_ids=list(range(NCORES)))
    out = np.empty((B, S, D), np.float32)
    for c in range(NCORES):
        b, r = c // 4, c % 4
        out[b, QS * r:QS * r + QS, :] = r_.results[c]["out"][::-1, :]
    return out

